# revision 1
# baseline (speedup 1.0000x reference)
"""Trainium2 Bass kernel for an LSTM encoder-decoder chatbot model.

Model: question -> embed -> LSTM(512) -> linear(256) = q_out
       answer[:, :256] -> embed -> concat(q_out) -> LSTM(512) -> linear(32000)
Output: logits [B=32, W=32000, STEPS=256] f32.

Sharding: all 8 cores run the full (replicated) encoder + decoder
recurrence; the dominant 512x32000 output projection is sharded
column-wise (vocab) across cores; each core emits [32, 4000, 256].

Matmul strategy: hidden state kept transposed (hT fp16 [128, 4x32])
as the PE stationary operand; weights stream as the moving operand in
fp16. Four col-tiled matmuls (tile_position=(0,32c)) run concurrently,
one per 512-unit gate block, so the gates land on all 128 PSUM
partitions [(block,b), 512=i|f|g|o] and the elementwise LSTM cell runs
full-width. Gate columns are host-permuted accordingly.
"""
import sys
import numpy as np

sys.path.insert(0, '/opt/trn_rl_repo')

import concourse.bass as bass  # noqa: E402
import concourse.bacc as bacc  # noqa: E402
import concourse.mybir as mybir  # noqa: E402
import concourse.tile as tile  # noqa: E402
from concourse.bass import IndirectOffsetOnAxis  # noqa: E402
from concourse.bass_utils import run_bass_kernel_spmd  # noqa: E402

F32 = mybir.dt.float32
F16 = mybir.dt.float16
I32DT = mybir.dt.int32
AF = mybir.ActivationFunctionType

W_VOCAB = 32000
EMB = 256
STEPS = 256
HID = 512
QOUT = 256
B = 32
LQ = 50
NCORES = 8
VSH = W_VOCAB // NCORES      # 4000 vocab rows per core
VPAD = 4096                   # padded to 32 tiles of 128
G = 4 * HID                   # 2048 gate columns
TBLK = 32                     # decoder steps per hs block (8 blocks)

_cache = {}


def _gate_perm():
    """Block layout [i|f|o|g]x128 per 128-unit block: new col
    j = 512*blk + 128*slot + u  <-  old row 512*gate + 128*blk + u,
    with slot order (i, f, o, g) so the three sigmoids are contiguous."""
    j = np.arange(G)
    blk, r = j // 512, j % 512
    slot, u = r // 128, r % 128
    old_gate = np.array([0, 1, 3, 2])[slot]
    return 512 * old_gate + 128 * blk + u


def build_program():
    nc = bacc.Bacc("TRN2", target_bir_lowering=False, debug=False,
                   num_devices=NCORES)

    def inp(name, shape, dt):
        return nc.dram_tensor(name, shape, dt, kind="ExternalInput").ap()

    q_idx = inp("q_idx", [13 * 128], I32DT)            # padded 1664
    a_idx = inp("a_idx", [STEPS * B], I32DT)           # 8192, t-major
    q_emb = inp("q_emb", [W_VOCAB, EMB], F16)
    a_emb = inp("a_emb", [W_VOCAB, EMB], F16)
    w_ihT_enc = inp("w_ihT_enc", [EMB, G], F16)        # permuted cols
    w_hhT_enc = inp("w_hhT_enc", [HID, G], F16)
    bias_enc = inp("bias_enc", [1, G], F16)
    w_ihAT = inp("w_ihAT", [EMB, G], F16)
    w_ihQT = inp("w_ihQT", [QOUT, G], F16)
    w_hhT_dec = inp("w_hhT_dec", [HID, G], F16)
    bias_dec = inp("bias_dec", [1, G], F16)
    q_lin_wT = inp("q_lin_wT", [HID, QOUT], F16)
    q_lin_b = inp("q_lin_b", [1, QOUT], F16)
    lin_wT = inp("lin_wT", [HID, VPAD], F16)           # per-core slice
    lin_b = inp("lin_b", [128, 32], F32)               # [u, mtile]
    i128f = inp("i128f", [128, 128], F32)
    i128h = inp("i128h", [128, 128], F16)
    i32h = inp("i32h", [32, 32], F16)
    ones1 = inp("ones1", [1, 32], F16)
    out = nc.dram_tensor("out", [B, VPAD, STEPS], F32,
                         kind="ExternalOutput").ap()

    with tile.TileContext(nc) as tc:
        _build(nc, tc, locals())
    nc.compile()
    return nc


def _build(nc, tc, t):
    from contextlib import ExitStack
    ctx = ExitStack()
    with ctx:
        _build_inner(nc, tc, t, ctx)


def _build_inner(nc, tc, t, ctx):
    # ---- pools -------------------------------------------------------
    wpool = ctx.enter_context(tc.tile_pool(name="weights", bufs=1))
    const = ctx.enter_context(tc.tile_pool(name="const", bufs=1))
    embp = ctx.enter_context(tc.tile_pool(name="embp", bufs=6))
    seqp = ctx.enter_context(tc.tile_pool(name="seqp", bufs=1))
    state = ctx.enter_context(tc.tile_pool(name="state", bufs=3))
    ew = ctx.enter_context(tc.tile_pool(name="ew", bufs=3))
    hsp = ctx.enter_context(tc.tile_pool(name="hsp", bufs=3))
    outp = ctx.enter_context(tc.tile_pool(name="outp", bufs=6))
    ps_g = ctx.enter_context(tc.tile_pool(name="ps_g", bufs=3, space="PSUM"))
    ps_tr = ctx.enter_context(tc.tile_pool(name="ps_tr", bufs=2, space="PSUM"))
    ps_p = ctx.enter_context(tc.tile_pool(name="ps_p", bufs=3, space="PSUM"))

    def load(pool, ap, dt=None, name=None):
        s = pool.tile(list(ap.shape), dt or ap.dtype, tag=name, name=name or 'ld')
        nc.sync.dma_start(s[:], ap[:])
        return s

    def loadc(pool, ap, name):
        p, cdim = ap.shape
        n = p // 128
        s = pool.tile([128, n * cdim], ap.dtype, tag=name, name=name)
        for k in range(n):
            nc.sync.dma_start(s[:, cdim * k:cdim * (k + 1)],
                              ap[128 * k:128 * (k + 1), :])
        def chunk(k, sl=slice(None)):
            base = cdim * k
            if sl == slice(None):
                return s[:, base:base + cdim]
            return s[:, base + sl.start:base + sl.stop]
        return chunk

    # ---- resident weights/constants ---------------------------------
    wih_e = loadc(wpool, t["w_ihT_enc"], "wih_e")     # 2 chunks [128,2048]
    whh_e = loadc(wpool, t["w_hhT_enc"], "whh_e")     # 4 chunks
    b_e = load(const, t["bias_enc"], name="b_e")
    wihA = loadc(wpool, t["w_ihAT"], "wihA")
    wihQ = loadc(wpool, t["w_ihQT"], "wihQ")
    whh_d = loadc(wpool, t["w_hhT_dec"], "whh_d")
    b_d = load(const, t["bias_dec"], name="b_d")
    qlw = loadc(wpool, t["q_lin_wT"], "qlw")          # 4 chunks [128,256]
    qlb = load(const, t["q_lin_b"], name="qlb")
    linw = loadc(wpool, t["lin_wT"], "linw")          # 4 chunks [128,4096]
    linb = load(const, t["lin_b"], name="linb")           # [128, 32] f32
    I128f = load(const, t["i128f"], name="I128f")
    I128h = load(const, t["i128h"], name="I128h")
    I32h = load(const, t["i32h"], name="I32h")
    ones = load(const, t["ones1"], name="ones")

    # index tiles
    qidx_sb = load(const, t["q_idx"].rearrange("(n p) -> n p", p=128)
                   .rearrange("n p -> p n"), name="qidx")   # [128, 13]
    aidx_sb = load(const, t["a_idx"].rearrange("(n p) -> n p", p=128)
                   .rearrange("n p -> p n"), name="aidx")   # [128, 64]

    # ---- embedding gather + transpose -> xT tiles --------------------
    def embed_T(table, idx_sb, ntiles, name):
        """gather rows (t-major) and transpose into xT [2 x [128, ntiles*128]] f16"""
        xT = [seqp.tile([128, ntiles * 128], F16, tag=f"{name}{k}", name=f"{name}{k}")
              for k in range(2)]
        for i in range(ntiles):
            rows = embp.tile([128, EMB], F16, tag="gather")
            nc.gpsimd.indirect_dma_start(
                out=rows[:], out_offset=None, in_=table[:],
                in_offset=IndirectOffsetOnAxis(ap=idx_sb[:, i:i + 1], axis=0))
            for k in range(2):
                p = ps_tr.tile([128, 128], F16, space="PSUM", tag="tr",
                               name="trp")
                nc.tensor.transpose(p[:], rows[:, 128 * k:128 * (k + 1)],
                                    I128h[:])
                nc.vector.tensor_copy(xT[k][:, 128 * i:128 * (i + 1)], p[:])
        return xT

    qT = embed_T(t["q_emb"], qidx_sb, 13, "qT")    # [256, 1664] f16
    # aT emission is deferred: tiles created now, per-tile gather+transpose
    # emitted interleaved into the encoder steps to fill PE chain gaps.
    aT = [seqp.tile([128, 64 * 128], F16, tag=f"aT{k}", name=f"aT{k}")
          for k in range(2)]

    def emit_aT(i):
        rows = embp.tile([128, EMB], F16, tag="gather", name="arows")
        nc.gpsimd.indirect_dma_start(
            out=rows[:], out_offset=None, in_=t["a_emb"][:],
            in_offset=IndirectOffsetOnAxis(ap=aidx_sb[:, i:i + 1], axis=0))
        for k in range(2):
            p = ps_tr.tile([128, 128], F16, space="PSUM", tag="tr",
                           name="trpa")
            nc.tensor.transpose(p[:], rows[:, 128 * k:128 * (k + 1)],
                                I128h[:])
            nc.vector.tensor_copy(aT[k][:, 128 * i:128 * (i + 1)], p[:])

    # ---- LSTM cell ---------------------------------------------------
    def step(hT, c_prev, seeds, wx_list, whh, has_h, want_hs):
        """One LSTM step, full-width col-tiled.

        seeds: list of (lhsT_ap[K,32], rhs_ap[K, 2048]) matmuls
        wx_list: list of (lhsT_ap, chunk_fn, k); whh: chunk accessor.
        """
        gp = ps_g.tile([128, 512], F32, space="PSUM", tag="gates")
        # rows = (lhsT_fn(sl), rhs_fn(sl)) emitted strip-innermost so the 4
        # col-strips run concurrently on the PE array
        rows = []
        for lhsT, rhs in seeds:
            rows.append((lambda sl, l=lhsT, r=rhs: (l, r[:, sl])))
        for lhsT, cf, k in wx_list:
            rows.append((lambda sl, l=lhsT, c2=cf, kk=k: (l, c2(kk, sl))))
        if has_h:
            for k in range(4):
                rows.append((lambda sl, kk=k: (hT[:, 32 * kk:32 * (kk + 1)],
                                               whh(kk, sl))))
        nrows = len(rows)
        for i, rowf in enumerate(rows):
            for c in range(4):
                sl = slice(512 * c, 512 * (c + 1))
                lhsT, rhs = rowf(sl)
                nc.tensor.matmul(gp[32 * c:32 * (c + 1), :], lhsT, rhs,
                                 start=(i == 0), stop=(i == nrows - 1),
                                 tile_position=(0, 32 * c))
        sig = ew.tile([128, 384], F32, tag="sig")   # i | f | o
        nc.scalar.activation(sig[:], gp[:, 0:384], AF.Sigmoid)
        gg = ew.tile([128, 128], F32, tag="g")
        nc.scalar.activation(gg[:], gp[:, 384:512], AF.Tanh)
        igg = ew.tile([128, 128], F32, tag="ig")
        nc.vector.tensor_mul(igg[:], sig[:, 0:128], gg[:])
        c_new = state.tile([128, 128], F32, tag="c")
        if c_prev is None:
            nc.vector.tensor_copy(c_new[:], igg[:])  # c0 = 0 -> c = i*g
        else:
            fc = ew.tile([128, 128], F32, tag="fc")
            nc.vector.tensor_mul(fc[:], sig[:, 128:256], c_prev[:])
            nc.vector.tensor_add(c_new[:], igg[:], fc[:])
        th = ew.tile([128, 128], F32, tag="th")
        nc.scalar.activation(th[:], c_new[:], AF.Tanh)
        h_new = ew.tile([128, 128], F16, tag="h")
        nc.vector.tensor_mul(h_new[:], sig[:, 256:384], th[:])
        trp = ps_tr.tile([128, 128], F16, space="PSUM", tag="tr", name="trh")
        nc.tensor.transpose(trp[:], h_new[:], I128h[:])
        hT_new = state.tile([128, 128], F16, tag="hT")
        nc.vector.tensor_copy(hT_new[:], trp[:])
        return hT_new, c_new

    # ---- encoder -----------------------------------------------------
    hT = None
    c = None
    a_emitted = 0
    for tt in range(LQ):
        sl32 = slice(32 * tt, 32 * (tt + 1))
        seeds = [(ones[:], b_e[:])]
        wx = [(qT[0][:, sl32], wih_e, 0),
              (qT[1][:, sl32], wih_e, 1)]
        hT, c = step(hT, c, seeds, wx, whh_e, has_h=(tt > 0), want_hs=False)
        want = (tt + 1) * 64 // LQ
        while a_emitted < want:
            emit_aT(a_emitted)
            a_emitted += 1
    while a_emitted < 64:
        emit_aT(a_emitted)
        a_emitted += 1

    # ---- q_out = h @ q_lin_w.T + b; then Qb = q_out @ w_ihQ.T + bias_dec
    qo_p_t = ps_p.tile([128, 512], F32, space="PSUM", tag="proj", name="qo_p")
    qo_p = qo_p_t[0:32, 0:QOUT]
    nc.tensor.matmul(qo_p[:], ones[:], qlb[:], start=True, stop=False)
    for k in range(4):
        nc.tensor.matmul(qo_p[:], hT[:, 32 * k:32 * (k + 1)],
                         qlw(k), start=False, stop=(k == 3))
    qo = seqp.tile([32, QOUT], F16, tag="qo_sb")
    nc.scalar.activation(qo[:], qo_p[:], AF.Identity)
    # transpose q_out [32,256] -> [256(2x128), 32] f16
    qoT = seqp.tile([128, 64], F16, tag="qoT")
    for k in range(2):
        p = ps_tr.tile([128, 128], F16, space="PSUM", tag="tr", name="trq")
        nc.tensor.transpose(p[:, 0:32], qo[:, 128 * k:128 * (k + 1)], I32h[:])
        nc.vector.tensor_copy(qoT[:, 32 * k:32 * (k + 1)], p[:, 0:32])
    # Qb [32, 2048] f16, quarter at a time (no col tiling, partition 0-31)
    qb = seqp.tile([32, G], F16, tag="qb")
    for qtr in range(4):
        sl = slice(512 * qtr, 512 * (qtr + 1))
        qp = ps_p.tile([128, 512], F32, space="PSUM", tag="proj", name="qp")[0:32, :]
        nc.tensor.matmul(qp[:], ones[:], b_d[:, sl], start=True, stop=False)
        for k in range(2):
            nc.tensor.matmul(qp[:], qoT[:, 32 * k:32 * (k + 1)],
                             wihQ(k, sl), start=False, stop=(k == 1))
        nc.scalar.activation(qb[:, sl], qp[:], AF.Identity)

    # ---- decoder + projection, software-pipelined --------------------
    # Block b's 32 vocab-tile projections are emitted one per step during
    # block b+1's recurrence, filling PE gaps in the chain-bound LSTM.
    out = t["out"]

    def proj_m(hs, blk, m):
        for s in range(TBLK * 32 // 512):
            pp = ps_p.tile([128, 512], F32, space="PSUM", tag="proj")
            for k in range(4):
                nc.tensor.matmul(
                    pp[:], linw(k, slice(128 * m, 128 * (m + 1))),
                    hs[:, TBLK * 32 * k + 512 * s:TBLK * 32 * k + 512 * (s + 1)],
                    start=(k == 0), stop=(k == 3))
            ot = outp.tile([128, 512], F32, tag="ot")
            if m % 2 == 0:
                nc.scalar.activation(ot[:], pp[:], AF.Identity,
                                     bias=linb[:, m:m + 1])
            else:
                nc.vector.tensor_scalar_add(ot[:], pp[:], linb[:, m:m + 1])
            nb = 512 // TBLK  # batches per sub-block
            dst = out[nb * s:nb * (s + 1), 128 * m:128 * (m + 1),
                      blk * TBLK:(blk + 1) * TBLK].rearrange("b w t -> w b t")
            nc.sync.dma_start(dst, ot[:].rearrange("w (b t) -> w b t", b=nb))

    hs_prev = None
    for blk in range(STEPS // TBLK):
        hs = hsp.tile([128, 4 * TBLK * 32], F16, tag="hs", name="hs")
        for dt in range(TBLK):
            tt = blk * TBLK + dt
            sl32 = slice(32 * tt, 32 * (tt + 1))
            seeds = [(I32h[:], qb[:])]
            wx = [(aT[0][:, sl32], wihA, 0),
                  (aT[1][:, sl32], wihA, 1)]
            hT, c = step(hT, c, seeds, wx, whh_d, has_h=True, want_hs=True)
            # scatter hT into the hs block: col (k*32*TBLK + b*TBLK + dt)
            dst = hs.rearrange("p (k b t) -> p k b t", k=4, b=32)[:, :, :, dt]
            nc.gpsimd.tensor_copy(dst, hT[:].rearrange("p (k b) -> p k b", k=4))
            if hs_prev is not None:
                proj_m(hs_prev, blk - 1, dt)
        hs_prev = hs
    for m in range(VPAD // 128):
        proj_m(hs_prev, STEPS // TBLK - 1, m)


def kernel(**inputs):
    inp = {k: np.asarray(v) for k, v in inputs.items()}
    if "prog" not in _cache:
        _cache["prog"] = build_program()
    nc = _cache["prog"]

    perm = _gate_perm()
    f16 = np.float16

    def prep_lstm(w_ih, w_hh, b_ih, b_hh):
        wihT = np.ascontiguousarray(w_ih.T[:, perm]).astype(f16)
        whhT = np.ascontiguousarray(w_hh.T[:, perm]).astype(f16)
        bias = (b_ih + b_hh)[perm][None, :].astype(f16)
        return wihT, whhT, bias

    wihT_e, whhT_e, b_e = prep_lstm(inp["q_lstm_w_ih"], inp["q_lstm_w_hh"],
                                    inp["q_lstm_b_ih"], inp["q_lstm_b_hh"])
    wihT_d, whhT_d, b_d = prep_lstm(inp["a_lstm_w_ih"], inp["a_lstm_w_hh"],
                                    inp["a_lstm_b_ih"], inp["a_lstm_b_hh"])
    wihAT = np.ascontiguousarray(wihT_d[:EMB])
    wihQT = np.ascontiguousarray(wihT_d[EMB:])

    q_idx = np.zeros(13 * 128, np.int32)
    q_idx[:B * LQ] = inp["question"].T.reshape(-1).astype(np.int32)
    a_idx = inp["answer"][:, :STEPS].T.reshape(-1).astype(np.int32)

    lin_w = inp["lin_w"].astype(np.float32)   # [32000, 512]
    lin_b = inp["lin_b"].astype(np.float32)

    base = {
        "q_idx": q_idx, "a_idx": a_idx,
        "q_emb": inp["q_emb_w"].astype(f16),
        "a_emb": inp["a_emb_w"].astype(f16),
        "w_ihT_enc": wihT_e, "w_hhT_enc": whhT_e, "bias_enc": b_e,
        "w_ihAT": wihAT, "w_ihQT": wihQT, "w_hhT_dec": whhT_d,
        "bias_dec": b_d,
        "q_lin_wT": np.ascontiguousarray(inp["q_lin_w"].T).astype(f16),
        "q_lin_b": inp["q_lin_b"][None, :].astype(f16),
        "i128f": np.eye(128, dtype=np.float32),
        "i128h": np.eye(128, dtype=f16),
        "i32h": np.eye(32, dtype=f16),
        "ones1": np.ones((1, 32), f16),
    }
    in_maps = []
    for core in range(NCORES):
        m = dict(base)
        sl = lin_w[VSH * core: VSH * (core + 1)]          # [4000, 512]
        slp = np.zeros((VPAD, HID), np.float32)
        slp[:VSH] = sl
        m["lin_wT"] = np.ascontiguousarray(slp.T).astype(f16)
        bp = np.zeros(VPAD, np.float32)
        bp[:VSH] = lin_b[VSH * core: VSH * (core + 1)]
        m["lin_b"] = np.ascontiguousarray(bp.reshape(32, 128).T)
        in_maps.append(m)

    _cache["in_maps"] = in_maps
    res = run_bass_kernel_spmd(nc, in_maps, core_ids=list(range(NCORES)))
    _cache["last_res"] = res
    out = np.concatenate(
        [res.results[i]["out"][:, :VSH, :] for i in range(NCORES)], axis=1)
    return out.astype(np.float32)


if __name__ == "__main__":
    import reference
    ins = reference.setup_inputs()
    ref = np.asarray(reference.reference(**ins))
    got = kernel(**{k: np.asarray(v) for k, v in ins.items()})
    err = np.abs(got - ref).max() / (np.abs(ref).max() + 1e-12)
    print("max abs err:", np.abs(got - ref).max(), "rel:", err)


def run_traced():
    nc = _cache["prog"]
    return run_bass_kernel_spmd(nc, _cache["in_maps"],
                                core_ids=list(range(NCORES)), trace=True)



# revision 5
# speedup vs baseline: 1.6385x; 1.6385x over previous
"""Trainium2 Bass kernel for an LSTM encoder-decoder chatbot model.

Model: question -> embed -> LSTM(512) -> linear(256) = q_out
       answer[:, :256] -> embed -> concat(q_out) -> LSTM(512) -> linear(32000)
Output: logits [B=32, W=32000, STEPS=256] f32.

Sharding: all 8 cores run the full (replicated) encoder + decoder
recurrence; the 512x32000 output projection is sharded column-wise
(vocab) across cores; each core emits a [4096, 8192] f16 tile that the
host reshapes to [32, 4000, 256] and bias-adds.

Matmul strategy (cost model charges out-free-size N per instruction,
independent of K/M): gates are computed TRANSPOSED — gate units on the
128 PSUM partitions, batch (32) on the free dim. Each step is 16
sequential per-bank accumulation groups x 7 matmuls (1 bias/q seed via
identity, 2 x-chunks, 4 h-chunks) of N=32, i.e. 3584 PE rows/step vs
14336 for the batch-on-partition layout. h emerges already transposed
(no per-step PE transpose), written straight into a [128, 4*512] hs
block that feeds the vocab projection as the moving operand.
"""
import sys
import numpy as np

sys.path.insert(0, '/opt/trn_rl_repo')

import concourse.bass as bass  # noqa: E402
import concourse.bacc as bacc  # noqa: E402
import concourse.mybir as mybir  # noqa: E402
import concourse.tile as tile  # noqa: E402
from concourse.bass import IndirectOffsetOnAxis  # noqa: E402
from concourse.bass_utils import run_bass_kernel_spmd  # noqa: E402

F32 = mybir.dt.float32
F16 = mybir.dt.float16
I32DT = mybir.dt.int32
AF = mybir.ActivationFunctionType

W_VOCAB = 32000
EMB = 256
STEPS = 256
HID = 512
QOUT = 256
B = 32
LQ = 50
NCORES = 8
VSH = W_VOCAB // NCORES       # 4000 vocab rows per core
VPAD = 4096                   # padded to 32 tiles of 128
G = 4 * HID                   # 2048 gate units
NMT = G // 128                # 16 gate tiles per step
BLK = 16                      # decoder steps per hs block
NBLK = STEPS // BLK           # 16 blocks
TOK = BLK * B                 # 512 tokens per block
NVT = VPAD // 128             # 32 vocab tiles per core

_cache = {}


def _gate_perm():
    """Permuted gate unit g^ = 512*grp + 128*blk + u with grp order
    (i, f, o, g) <- old row 512*old_gate + 128*blk + u. Gate tile
    mt = g^//128 = 4*grp + blk, so PSUM cols [0:128)=i, [128:256)=f,
    [256:384)=o, [384:512)=g, each laid [p=u, 32*blk + b] — identical
    to the h/c layout [u, 32*blk + b]."""
    j = np.arange(G)
    grp, r = j // 512, j % 512
    blk, u = r // 128, r % 128
    old_gate = np.array([0, 1, 3, 2])[grp]
    return 512 * old_gate + 128 * blk + u


def build_program():
    nc = bacc.Bacc("TRN2", target_bir_lowering=False, debug=False,
                   num_devices=NCORES)

    def inp(name, shape, dt):
        return nc.dram_tensor(name, shape, dt, kind="ExternalInput").ap()

    q_idx = inp("q_idx", [13 * 128], I32DT)            # padded 1664, t-major
    a_idx = inp("a_idx", [64 * 128], I32DT)            # 8192, t-major
    q_emb = inp("q_emb", [W_VOCAB, EMB], F16)
    a_emb = inp("a_emb", [W_VOCAB, EMB], F16)
    wihT_e = inp("wihT_e", [EMB, G], F16)              # permuted cols
    whhT_e = inp("whhT_e", [HID, G], F16)
    beT = inp("beT", [128, 512], F16)                  # enc bias, tiled [p, 32mt+b]
    wihAT = inp("wihAT", [EMB, G], F16)
    wihQT = inp("wihQT", [QOUT, G], F16)
    whhT_d = inp("whhT_d", [HID, G], F16)
    bdT = inp("bdT", [128, 512], F16)                  # dec bias, tiled
    qlwT = inp("qlwT", [HID, QOUT], F16)
    qlbT = inp("qlbT", [128, 64], F16)                 # q_lin_b tiled [p, 32qt+b]
    lin_wT = inp("lin_wT", [HID, VPAD], F16)           # per-core slice
    i128h = inp("i128h", [128, 128], F16)
    out = nc.dram_tensor("out", [VPAD, NBLK * TOK], F16,
                         kind="ExternalOutput").ap()

    with tile.TileContext(nc) as tc:
        _build(nc, tc, locals())
    nc.compile()
    return nc


def _build(nc, tc, t):
    from contextlib import ExitStack
    ctx = ExitStack()
    with ctx:
        _build_inner(nc, tc, t, ctx)


def _build_inner(nc, tc, t, ctx):
    # ---- pools -------------------------------------------------------
    wpool = ctx.enter_context(tc.tile_pool(name="weights", bufs=1))
    const = ctx.enter_context(tc.tile_pool(name="const", bufs=1))
    embp = ctx.enter_context(tc.tile_pool(name="embp", bufs=4))
    seqp = ctx.enter_context(tc.tile_pool(name="seqp", bufs=1))
    state = ctx.enter_context(tc.tile_pool(name="state", bufs=2))
    ew = ctx.enter_context(tc.tile_pool(name="ew", bufs=2))
    hsp = ctx.enter_context(tc.tile_pool(name="hsp", bufs=2))
    outp = ctx.enter_context(tc.tile_pool(name="outp", bufs=2))
    ps_g = ctx.enter_context(tc.tile_pool(name="ps_g", bufs=2, space="PSUM"))
    ps_tr = ctx.enter_context(tc.tile_pool(name="ps_tr", bufs=2, space="PSUM"))
    ps_p = ctx.enter_context(tc.tile_pool(name="ps_p", bufs=2, space="PSUM"))

    def load(pool, ap, dt=None, name=None):
        s = pool.tile(list(ap.shape), dt or ap.dtype, tag=name, name=name or 'ld')
        nc.sync.dma_start(s[:], ap[:])
        return s

    def loadc(pool, ap, name):
        p, cdim = ap.shape
        n = p // 128
        s = pool.tile([128, n * cdim], ap.dtype, tag=name, name=name)
        for k in range(n):
            nc.sync.dma_start(s[:, cdim * k:cdim * (k + 1)],
                              ap[128 * k:128 * (k + 1), :])

        def chunk(k, sl=slice(None)):
            base = cdim * k
            if sl == slice(None):
                return s[:, base:base + cdim]
            return s[:, base + sl.start:base + sl.stop]
        return chunk

    # ---- resident weights/constants ---------------------------------
    wih_e = loadc(wpool, t["wihT_e"], "wih_e")       # 2 chunks [128, 2048]
    whh_e = loadc(wpool, t["whhT_e"], "whh_e")       # 4 chunks
    beT = load(const, t["beT"], name="beT")
    wihA = loadc(wpool, t["wihAT"], "wihA")
    wihQ = loadc(wpool, t["wihQT"], "wihQ")
    whh_d = loadc(wpool, t["whhT_d"], "whh_d")
    bdT = load(const, t["bdT"], name="bdT")
    qlw = loadc(wpool, t["qlwT"], "qlw")             # 4 chunks [128, 256]
    qlbT = load(const, t["qlbT"], name="qlbT")
    linw = loadc(wpool, t["lin_wT"], "linw")         # 4 chunks [128, 4096]
    I128h = load(const, t["i128h"], name="I128h")

    qidx_sb = load(const, t["q_idx"].rearrange("(n p) -> n p", p=128)
                   .rearrange("n p -> p n"), name="qidx")   # [128, 13]
    aidx_sb = load(const, t["a_idx"].rearrange("(n p) -> n p", p=128)
                   .rearrange("n p -> p n"), name="aidx")   # [128, 64]

    # ---- embedding gather + transpose helpers ------------------------
    # xT tiles: [128 = emb unit within chunk, token] f16, token t-major.
    qT = [seqp.tile([128, 13 * 128], F16, tag=f"qT{k}", name=f"qT{k}")
          for k in range(2)]
    aT = [seqp.tile([128, 64 * 128], F16, tag=f"aT{k}", name=f"aT{k}")
          for k in range(2)]

    def gather(table, idx_sb, i, name):
        rows = embp.tile([128, EMB], F16, tag="gather", name=name)
        nc.gpsimd.indirect_dma_start(
            out=rows[:], out_offset=None, in_=table[:],
            in_offset=IndirectOffsetOnAxis(ap=idx_sb[:, i:i + 1], axis=0))
        return rows

    def transpose_into(xT, rows, i, k):
        p = ps_tr.tile([128, 128], F16, space="PSUM", tag="tr", name="trp")
        nc.tensor.transpose(p[:], rows[:, 128 * k:128 * (k + 1)], I128h[:])
        nc.vector.tensor_copy(xT[k][:, 128 * i:128 * (i + 1)], p[:])

    # q side fully upfront (needed from encoder step 0; also ramps PE)
    for i in range(13):
        rows = gather(t["q_emb"], qidx_sb, i, "qrows")
        for k in range(2):
            transpose_into(qT, rows, i, k)

    # a side: gathers + transposes interleaved into encoder/early decoder
    a_rows = [None] * 64
    a_gathered = 0
    a_transposed = 0

    def emit_a_gather():
        nonlocal a_gathered
        if a_gathered < 64:
            a_rows[a_gathered] = gather(t["a_emb"], aidx_sb, a_gathered,
                                        "arows")
            a_gathered += 1

    def emit_a_transpose():
        nonlocal a_transposed
        if a_transposed < 2 * 64 and a_transposed < 2 * a_gathered:
            i, k = divmod(a_transposed, 2)
            transpose_into(aT, a_rows[i], i, k)
            a_transposed += 1

    # ---- one LSTM step, transposed-gates layout ----------------------
    # Returns (c_new, h_getter) where the new h^T lives either in a state
    # tile (encoder) or inside the current hs block (decoder).
    def step(seed_sb, xTt, wih, whh, h_ap, c_prev, h_dst_3d):
        """seed_sb: [128, 512] f16 SBUF (bias [+ q proj]), seeded via I128.
        xTt: per-chunk rhs aps ([128, 32] each) for the 2 x chunks.
        h_ap(kc): rhs ap [128, 32] for h chunk kc, or None (t=0 encoder).
        c_prev: [128, 128] f32 SBUF tile or None.
        h_dst_3d: [p, 4, 32] view where the DVE writes h^T.
        """
        gp = ps_g.tile([128, 512], F32, space="PSUM", tag="gates",
                       name="gates")
        has_h = h_ap is not None
        for mt in range(NMT):
            sl = slice(32 * mt, 32 * (mt + 1))
            msl = slice(128 * mt, 128 * (mt + 1))
            n_mm = 3 + (4 if has_h else 0)
            i_mm = 0
            nc.tensor.matmul(gp[:, sl], I128h[:], seed_sb[:, sl],
                             start=True, stop=(n_mm == 1),
                             skip_group_check=True)
            i_mm += 1
            for kx in range(2):
                nc.tensor.matmul(gp[:, sl], wih(kx, msl), xTt[kx],
                                 start=False, stop=(i_mm == n_mm - 1),
                                 skip_group_check=True)
                i_mm += 1
            if has_h:
                for kc in range(4):
                    nc.tensor.matmul(gp[:, sl], whh(kc, msl), h_ap(kc),
                                     start=False, stop=(i_mm == n_mm - 1),
                                     skip_group_check=True)
                    i_mm += 1
        # cell (i | f | o | g in cols 0:128 | 128:256 | 256:384 | 384:512)
        sig = ew.tile([128, 384], F16, tag="sig", name="sig")
        nc.scalar.activation(sig[:], gp[:, 0:384], AF.Sigmoid)
        gg = ew.tile([128, 128], F16, tag="gg", name="gg")
        nc.scalar.activation(gg[:], gp[:, 384:512], AF.Tanh)
        c_new = state.tile([128, 128], F32, tag="c", name="c")
        if c_prev is None:
            nc.vector.tensor_mul(c_new[:], sig[:, 0:128], gg[:])
        else:
            fc = ew.tile([128, 128], F32, tag="fc", name="fc")
            nc.gpsimd.tensor_mul(fc[:], sig[:, 128:256], c_prev[:])
            igg = ew.tile([128, 128], F16, tag="igg", name="igg")
            nc.vector.tensor_mul(igg[:], sig[:, 0:128], gg[:])
            nc.vector.tensor_add(c_new[:], igg[:], fc[:])
        th = ew.tile([128, 128], F16, tag="th", name="th")
        nc.scalar.activation(th[:], c_new[:], AF.Tanh)
        o3 = sig[:, 256:384].rearrange("p (k c) -> p k c", k=4)
        th3 = th[:].rearrange("p (k c) -> p k c", k=4)
        nc.vector.tensor_mul(h_dst_3d, o3, th3)
        return c_new

    # ---- encoder -----------------------------------------------------
    hT_enc = None
    c = None
    for tt in range(LQ):
        emit_a_gather()
        if tt % 3 == 2:
            emit_a_gather()
        xTt = [qT[k][:, 32 * tt:32 * (tt + 1)] for k in range(2)]
        if hT_enc is None:
            h_ap = None
        else:
            hp = hT_enc
            h_ap = (lambda kc, hp=hp: hp[:, 32 * kc:32 * (kc + 1)])
        h_new = state.tile([128, 128], F16, tag="hT", name="hT")
        c = step(beT, xTt, wih_e, whh_e, h_ap, c,
                 h_new[:].rearrange("p (k c) -> p k c", k=4))
        hT_enc = h_new
        emit_a_transpose()
        emit_a_transpose()

    # ---- q_out^T and qb^T -------------------------------------------
    # qoT [256, 32] = q_lin_w @ h + q_lin_b, as 2 tiles of [128, 32]
    qo_ps = ps_p.tile([128, 512], F32, space="PSUM", tag="proj", name="qo_ps")
    for qt in range(2):
        sl = slice(32 * qt, 32 * (qt + 1))
        nc.tensor.matmul(qo_ps[:, sl], I128h[:], qlbT[:, sl],
                         start=True, stop=False, skip_group_check=True)
        for kc in range(4):
            nc.tensor.matmul(qo_ps[:, sl],
                             qlw(kc, slice(128 * qt, 128 * (qt + 1))),
                             hT_enc[:, 32 * kc:32 * (kc + 1)],
                             start=False, stop=(kc == 3),
                             skip_group_check=True)
    qoT = seqp.tile([128, 64], F16, tag="qoT", name="qoT")
    nc.scalar.activation(qoT[:], qo_ps[:, 0:64], AF.Identity)
    # qbT [128, 512] f16: (W_ihQ @ q_out + bias_dec) tiled [p, 32mt+b]
    qb_ps = ps_g.tile([128, 512], F32, space="PSUM", tag="gates", name="qb_ps")
    for mt in range(NMT):
        sl = slice(32 * mt, 32 * (mt + 1))
        msl = slice(128 * mt, 128 * (mt + 1))
        nc.tensor.matmul(qb_ps[:, sl], I128h[:], bdT[:, sl],
                         start=True, stop=False, skip_group_check=True)
        for kq in range(2):
            nc.tensor.matmul(qb_ps[:, sl], wihQ(kq, msl),
                             qoT[:, 32 * kq:32 * (kq + 1)],
                             start=False, stop=(kq == 1),
                             skip_group_check=True)
    qbT = seqp.tile([128, 512], F16, tag="qbT", name="qbT")
    nc.scalar.activation(qbT[:], qb_ps[:], AF.Identity)

    # ---- decoder + projection, software-pipelined --------------------
    out = t["out"]
    evac_n = 0

    def proj_vt(hs_prev, kb_prev, vt, stage):
        nonlocal evac_n
        pp = ps_p.tile([128, 512], F32, space="PSUM", tag="proj", name="pp")
        for kc in range(4):
            nc.tensor.matmul(pp[:], linw(kc, slice(128 * vt, 128 * (vt + 1))),
                             hs_prev[:, 512 * kc:512 * (kc + 1)],
                             start=(kc == 0), stop=(kc == 3),
                             skip_group_check=True)
        q = vt % 4
        dst = stage[:, 512 * q:512 * (q + 1)]
        if evac_n % 2 == 0:
            nc.scalar.activation(dst, pp[:], AF.Identity)
        else:
            nc.vector.tensor_copy(dst, pp[:])
        evac_n += 1
        if q == 3:
            ddst = out[128 * (vt - 3):128 * (vt + 1),
                       TOK * kb_prev:TOK * (kb_prev + 1)] \
                .rearrange("(v p) c -> p v c", v=4)
            nc.sync.dma_start(ddst, stage[:].rearrange("p (v c) -> p v c", v=4))

    hs_prev = None
    stage = None
    # h accessor: encoder final state, layout [u, 32*kc + b]
    h_acc = (lambda kc, hp=hT_enc: hp[:, 32 * kc:32 * (kc + 1)])
    for kb in range(NBLK):
        hs = hsp.tile([128, 4 * TOK], F16, tag="hs", name="hs")
        for st in range(BLK):
            xTt = [aT[k][:, 32 * (BLK * kb + st):32 * (BLK * kb + st) + 32]
                   for k in range(2)]
            hd3 = hs[:].rearrange("p (k c) -> p k c", k=4)[:, :, 32 * st:
                                                          32 * (st + 1)]
            c = step(qbT, xTt, wihA, whh_d, h_acc, c, hd3)
            # h(t) now lives at hs cols (512*kc + 32*st + b)
            h_acc = (lambda kc, hp=hs, s=st: hp[:, 512 * kc + 32 * s:
                                                512 * kc + 32 * s + 32])
            if hs_prev is not None:
                if st % 2 == 0:
                    stage = outp.tile([128, 2048], F16, tag="ostage",
                                      name="ostage")
                proj_vt(hs_prev, kb - 1, 2 * st, stage)
                proj_vt(hs_prev, kb - 1, 2 * st + 1, stage)
            else:
                emit_a_transpose()
                emit_a_transpose()
                emit_a_transpose()
        hs_prev = hs
    # tail: last block's projection
    for vt in range(NVT):
        if vt % 4 == 0:
            stage = outp.tile([128, 2048], F16, tag="ostage", name="ostage")
        proj_vt(hs_prev, NBLK - 1, vt, stage)


def kernel(**inputs):
    inp = {k: np.asarray(v) for k, v in inputs.items()}
    if "prog" not in _cache:
        _cache["prog"] = build_program()
    nc = _cache["prog"]

    perm = _gate_perm()
    f16 = np.float16

    def tile_bias(bias_perm, n):
        # [n*128] -> [128, n*32]: out[p, 32*mt + b] = bias[128*mt + p]
        return np.repeat(bias_perm.reshape(n, 128).T[:, :, None], 32,
                         axis=2).reshape(128, n * 32).astype(f16)

    def prep_lstm(w_ih, w_hh, b_ih, b_hh):
        wihT = np.ascontiguousarray(w_ih.T[:, perm]).astype(f16)
        whhT = np.ascontiguousarray(w_hh.T[:, perm]).astype(f16)
        bias = (b_ih + b_hh)[perm]
        return wihT, whhT, tile_bias(bias, NMT)

    wihT_e, whhT_e, beT = prep_lstm(inp["q_lstm_w_ih"], inp["q_lstm_w_hh"],
                                    inp["q_lstm_b_ih"], inp["q_lstm_b_hh"])
    wihT_d, whhT_d, bdT = prep_lstm(inp["a_lstm_w_ih"], inp["a_lstm_w_hh"],
                                    inp["a_lstm_b_ih"], inp["a_lstm_b_hh"])
    wihAT = np.ascontiguousarray(wihT_d[:EMB])
    wihQT = np.ascontiguousarray(wihT_d[EMB:])

    q_idx = np.zeros(13 * 128, np.int32)
    q_idx[:B * LQ] = inp["question"].T.reshape(-1).astype(np.int32)
    a_idx = inp["answer"][:, :STEPS].T.reshape(-1).astype(np.int32)

    lin_w = inp["lin_w"].astype(np.float32)   # [32000, 512]
    lin_b = inp["lin_b"].astype(np.float32)

    base = {
        "q_idx": q_idx, "a_idx": a_idx,
        "q_emb": inp["q_emb_w"].astype(f16),
        "a_emb": inp["a_emb_w"].astype(f16),
        "wihT_e": wihT_e, "whhT_e": whhT_e, "beT": beT,
        "wihAT": wihAT, "wihQT": wihQT, "whhT_d": whhT_d, "bdT": bdT,
        "qlwT": np.ascontiguousarray(inp["q_lin_w"].T).astype(f16),
        "qlbT": tile_bias(inp["q_lin_b"].astype(np.float64), 2),
        "i128h": np.eye(128, dtype=f16),
    }
    in_maps = []
    for core in range(NCORES):
        m = dict(base)
        sl = lin_w[VSH * core: VSH * (core + 1)]          # [4000, 512]
        slp = np.zeros((VPAD, HID), np.float32)
        slp[:VSH] = sl
        m["lin_wT"] = np.ascontiguousarray(slp.T).astype(f16)
        in_maps.append(m)

    _cache["in_maps"] = in_maps
    res = run_bass_kernel_spmd(nc, in_maps, core_ids=list(range(NCORES)))
    _cache["last_res"] = res

    final = np.empty((B, W_VOCAB, STEPS), np.float32)
    for core in range(NCORES):
        arr = np.asarray(res.results[core]["out"], dtype=np.float32)
        # [4096, 8192] col = 512*kb + 32*st + b; t = 16*kb + st
        arr = arr.reshape(VPAD, NBLK, BLK, B).transpose(3, 0, 1, 2) \
            .reshape(B, VPAD, STEPS)
        final[:, VSH * core:VSH * (core + 1), :] = arr[:, :VSH, :]
    final += lin_b[None, :, None]
    return final


if __name__ == "__main__":
    import reference
    ins = reference.setup_inputs()
    ref = np.asarray(reference.reference(**ins))
    got = kernel(**{k: np.asarray(v) for k, v in ins.items()})
    err = np.abs(got - ref).max() / (np.abs(ref).max() + 1e-12)
    print("max abs err:", np.abs(got - ref).max(), "rel:", err)


def run_traced():
    nc = _cache["prog"]
    return run_bass_kernel_spmd(nc, _cache["in_maps"],
                                core_ids=list(range(NCORES)), trace=True)


# revision 9
# speedup vs baseline: 1.9066x; 1.1636x over previous
"""Trainium2 Bass kernel for an LSTM encoder-decoder chatbot model.

Model: question -> embed -> LSTM(512) -> linear(256) = q_out
       answer[:, :256] -> embed -> concat(q_out) -> LSTM(512) -> linear(32000)
Output: logits [B=32, W=32000, STEPS=256] f32.

Sharding: all 8 cores run the full (replicated) encoder + decoder
recurrence; the 512x32000 output projection is sharded column-wise
(vocab) across cores; each core emits a [4096, 8192] f16 tile that the
host reshapes to [32, 4000, 256] and bias-adds.

Matmul strategy (cost model charges out-free-size N per instruction,
independent of K/M): gates are computed TRANSPOSED — gate units on the
128 PSUM partitions, batch (32) on the free dim. Each step is 16
sequential per-bank accumulation groups x 7 matmuls (1 bias/q seed via
identity, 2 x-chunks, 4 h-chunks) of N=32, i.e. 3584 PE rows/step vs
14336 for the batch-on-partition layout. h emerges already transposed
(no per-step PE transpose), written straight into a [128, 4*512] hs
block that feeds the vocab projection as the moving operand.
"""
import sys
import numpy as np

sys.path.insert(0, '/opt/trn_rl_repo')

import concourse.bass as bass  # noqa: E402
import concourse.bacc as bacc  # noqa: E402
import concourse.mybir as mybir  # noqa: E402
import concourse.tile as tile  # noqa: E402
from concourse.bass import IndirectOffsetOnAxis  # noqa: E402
from concourse.bass_utils import run_bass_kernel_spmd  # noqa: E402

F32 = mybir.dt.float32
F16 = mybir.dt.float16
I32DT = mybir.dt.int32
AF = mybir.ActivationFunctionType

W_VOCAB = 32000
EMB = 256
STEPS = 256
HID = 512
QOUT = 256
B = 32
LQ = 50
NCORES = 8
VSH = W_VOCAB // NCORES       # 4000 vocab rows per core
VPAD = 4096                   # padded to 32 tiles of 128
G = 4 * HID                   # 2048 gate units
NMT = G // 128                # 16 gate tiles per step
BLK = 16                      # decoder steps per hs block
NBLK = STEPS // BLK           # 16 blocks
TOK = BLK * B                 # 512 tokens per block
NVT = VPAD // 128             # 32 vocab tiles per core

_cache = {}


def _gate_perm():
    """Permuted gate unit g^ = 512*grp + 128*blk + u with grp order
    (i, f, o, g) <- old row 512*old_gate + 128*blk + u. Gate tile
    mt = g^//128 = 4*grp + blk, so PSUM cols [0:128)=i, [128:256)=f,
    [256:384)=o, [384:512)=g, each laid [p=u, 32*blk + b] — identical
    to the h/c layout [u, 32*blk + b]."""
    j = np.arange(G)
    grp, r = j // 512, j % 512
    blk, u = r // 128, r % 128
    old_gate = np.array([0, 1, 3, 2])[grp]
    return 512 * old_gate + 128 * blk + u


def build_program():
    nc = bacc.Bacc("TRN2", target_bir_lowering=False, debug=False,
                   num_devices=NCORES)

    def inp(name, shape, dt):
        return nc.dram_tensor(name, shape, dt, kind="ExternalInput").ap()

    q_idx = inp("q_idx", [13 * 128], I32DT)            # padded 1664, t-major
    a_idx = inp("a_idx", [64 * 128], I32DT)            # 8192, t-major
    q_emb = inp("q_emb", [W_VOCAB, EMB], F16)
    a_emb = inp("a_emb", [W_VOCAB, EMB], F16)
    wihT_e = inp("wihT_e", [EMB, G], F16)              # permuted cols
    whhT_e = inp("whhT_e", [HID, G], F16)
    beT = inp("beT", [128, 512], F16)                  # enc bias, tiled [p, 32mt+b]
    wihAT = inp("wihAT", [EMB, G], F16)
    wihQT = inp("wihQT", [QOUT, G], F16)
    whhT_d = inp("whhT_d", [HID, G], F16)
    bdT = inp("bdT", [128, 512], F16)                  # dec bias, tiled
    qlwT = inp("qlwT", [HID, QOUT], F16)
    qlbT = inp("qlbT", [128, 64], F16)                 # q_lin_b tiled [p, 32qt+b]
    lin_wT = inp("lin_wT", [HID, VPAD], F16)           # per-core slice
    i128h = inp("i128h", [128, 128], F16)
    out = nc.dram_tensor("out", [VPAD, NBLK * TOK], F16,
                         kind="ExternalOutput").ap()

    with tile.TileContext(nc) as tc:
        _build(nc, tc, locals())
    nc.compile()
    return nc


def _build(nc, tc, t):
    from contextlib import ExitStack
    ctx = ExitStack()
    with ctx:
        _build_inner(nc, tc, t, ctx)


def _build_inner(nc, tc, t, ctx):
    # ---- pools -------------------------------------------------------
    wpool = ctx.enter_context(tc.tile_pool(name="weights", bufs=1))
    const = ctx.enter_context(tc.tile_pool(name="const", bufs=1))
    embp = ctx.enter_context(tc.tile_pool(name="embp", bufs=4))
    seqp = ctx.enter_context(tc.tile_pool(name="seqp", bufs=1))
    state = ctx.enter_context(tc.tile_pool(name="state", bufs=2))
    ew = ctx.enter_context(tc.tile_pool(name="ew", bufs=2))
    hsp = ctx.enter_context(tc.tile_pool(name="hsp", bufs=3))
    outp = ctx.enter_context(tc.tile_pool(name="outp", bufs=3))
    ps_g = ctx.enter_context(tc.tile_pool(name="ps_g", bufs=2, space="PSUM"))
    ps_tr = ctx.enter_context(tc.tile_pool(name="ps_tr", bufs=1, space="PSUM"))
    ps_p = ctx.enter_context(tc.tile_pool(name="ps_p", bufs=3, space="PSUM"))

    def load(pool, ap, dt=None, name=None):
        s = pool.tile(list(ap.shape), dt or ap.dtype, tag=name, name=name or 'ld')
        nc.sync.dma_start(s[:], ap[:])
        return s

    def loadc(pool, ap, name):
        p, cdim = ap.shape
        n = p // 128
        s = pool.tile([128, n * cdim], ap.dtype, tag=name, name=name)
        for k in range(n):
            nc.sync.dma_start(s[:, cdim * k:cdim * (k + 1)],
                              ap[128 * k:128 * (k + 1), :])

        def chunk(k, sl=slice(None)):
            base = cdim * k
            if sl == slice(None):
                return s[:, base:base + cdim]
            return s[:, base + sl.start:base + sl.stop]
        return chunk

    # ---- resident weights/constants ---------------------------------
    wih_e = loadc(wpool, t["wihT_e"], "wih_e")       # 2 chunks [128, 2048]
    whh_e = loadc(wpool, t["whhT_e"], "whh_e")       # 4 chunks
    beT = load(const, t["beT"], name="beT")
    wihA = loadc(wpool, t["wihAT"], "wihA")
    wihQ = loadc(wpool, t["wihQT"], "wihQ")
    whh_d = loadc(wpool, t["whhT_d"], "whh_d")
    bdT = load(const, t["bdT"], name="bdT")
    qlw = loadc(wpool, t["qlwT"], "qlw")             # 4 chunks [128, 256]
    qlbT = load(const, t["qlbT"], name="qlbT")
    linw = loadc(wpool, t["lin_wT"], "linw")         # 4 chunks [128, 4096]
    I128h = load(const, t["i128h"], name="I128h")

    qidx_sb = load(const, t["q_idx"].rearrange("(n p) -> n p", p=128)
                   .rearrange("n p -> p n"), name="qidx")   # [128, 13]
    aidx_sb = load(const, t["a_idx"].rearrange("(n p) -> n p", p=128)
                   .rearrange("n p -> p n"), name="aidx")   # [128, 64]

    # ---- embedding gather + transpose helpers ------------------------
    # xT tiles: [128 = emb unit within chunk, token] f16, token t-major.
    qT = [seqp.tile([128, 13 * 128], F16, tag=f"qT{k}", name=f"qT{k}")
          for k in range(2)]
    aT = [seqp.tile([128, 64 * 128], F16, tag=f"aT{k}", name=f"aT{k}")
          for k in range(2)]

    def gather(table, idx_sb, i, name):
        rows = embp.tile([128, EMB], F16, tag="gather", name=name)
        nc.gpsimd.indirect_dma_start(
            out=rows[:], out_offset=None, in_=table[:],
            in_offset=IndirectOffsetOnAxis(ap=idx_sb[:, i:i + 1], axis=0))
        return rows

    def transpose_into(xT, rows, i, k):
        p = ps_tr.tile([128, 128], F16, space="PSUM", tag="tr", name="trp")
        nc.tensor.transpose(p[:], rows[:, 128 * k:128 * (k + 1)], I128h[:])
        nc.vector.tensor_copy(xT[k][:, 128 * i:128 * (i + 1)], p[:])

    # q side fully upfront (needed from encoder step 0; also ramps PE)
    for i in range(13):
        rows = gather(t["q_emb"], qidx_sb, i, "qrows")
        for k in range(2):
            transpose_into(qT, rows, i, k)

    # a side: gathers + transposes interleaved into encoder/early decoder
    a_rows = [None] * 64
    a_gathered = 0
    a_transposed = 0

    def emit_a_gather():
        nonlocal a_gathered
        if a_gathered < 64:
            a_rows[a_gathered] = gather(t["a_emb"], aidx_sb, a_gathered,
                                        "arows")
            a_gathered += 1

    def emit_a_transpose():
        nonlocal a_transposed
        if a_transposed < 2 * 64 and a_transposed < 2 * a_gathered:
            i, k = divmod(a_transposed, 2)
            transpose_into(aT, a_rows[i], i, k)
            a_transposed += 1

    # ---- one LSTM step, transposed-gates layout ----------------------
    # Returns (c_new, h_getter) where the new h^T lives either in a state
    # tile (encoder) or inside the current hs block (decoder).
    def step(seed_sb, xTt, wih, whh, h_ap, c_prev, h_dst_3d):
        """seed_sb: [128, 512] f16 SBUF (bias [+ q proj]), seeded via I128.
        xTt: per-chunk rhs aps ([128, 32] each) for the 2 x chunks.
        h_ap(kc): rhs ap [128, 32] for h chunk kc, or None (t=0 encoder).
        c_prev: [128, 128] f32 SBUF tile or None.
        h_dst_3d: [p, 4, 32] view where the DVE writes h^T.
        """
        # two half-banks so the cell can start after the i,f tiles:
        # bank A cols = i (0:128) | f (128:256); bank B = g (0:128) | o
        # (128:256). PSUM column of permuted gate tile mt:
        #   mt 0..7  (i,f) -> A[:, 32*mt]
        #   mt 12..15 (g)  -> B[:, 32*(mt-12)]
        #   mt 8..11  (o)  -> B[:, 128 + 32*(mt-8)]
        gA = ps_g.tile([128, 256], F32, space="PSUM", tag="gA", name="gA")
        gB = ps_g.tile([128, 256], F32, space="PSUM", tag="gB", name="gB")
        has_h = h_ap is not None

        def emit_tile(gp, col, mt):
            sl = slice(col, col + 32)
            msl = slice(128 * mt, 128 * (mt + 1))
            n_mm = 3 + (4 if has_h else 0)
            i_mm = 0
            nc.tensor.matmul(gp[:, sl], I128h[:], seed_sb[:, 32 * mt:
                                                          32 * mt + 32],
                             start=True, stop=(n_mm == 1),
                             skip_group_check=True)
            i_mm += 1
            for kx in range(2):
                nc.tensor.matmul(gp[:, sl], wih(kx, msl), xTt[kx],
                                 start=False, stop=(i_mm == n_mm - 1),
                                 skip_group_check=True)
                i_mm += 1
            if has_h:
                for kc in range(4):
                    nc.tensor.matmul(gp[:, sl], whh(kc, msl), h_ap(kc),
                                     start=False, stop=(i_mm == n_mm - 1),
                                     skip_group_check=True)
                    i_mm += 1

        for mt in range(8):                      # i, f
            emit_tile(gA, 32 * mt, mt)
        for mt in range(12, 16):                 # g first in bank B
            emit_tile(gB, 32 * (mt - 12), mt)
        for mt in range(8, 12):                  # o last
            emit_tile(gB, 128 + 32 * (mt - 8), mt)

        sif = ew.tile([128, 256], F16, tag="sif", name="sif")
        nc.scalar.activation(sif[:], gA[:], AF.Sigmoid)
        gg = ew.tile([128, 128], F16, tag="gg", name="gg")
        nc.scalar.activation(gg[:], gB[:, 0:128], AF.Tanh)
        so = ew.tile([128, 128], F16, tag="so", name="so")
        nc.scalar.activation(so[:], gB[:, 128:256], AF.Sigmoid)
        c_new = state.tile([128, 128], F32, tag="c", name="c")
        if c_prev is None:
            nc.vector.tensor_mul(c_new[:], sif[:, 0:128], gg[:])
        else:
            fc = ew.tile([128, 128], F32, tag="fc", name="fc")
            nc.gpsimd.tensor_mul(fc[:], sif[:, 128:256], c_prev[:])
            igg = ew.tile([128, 128], F16, tag="igg", name="igg")
            nc.vector.tensor_mul(igg[:], sif[:, 0:128], gg[:])
            nc.vector.tensor_add(c_new[:], igg[:], fc[:])
        th = ew.tile([128, 128], F16, tag="th", name="th")
        nc.scalar.activation(th[:], c_new[:], AF.Tanh)
        o3 = so[:].rearrange("p (k c) -> p k c", k=4)
        th3 = th[:].rearrange("p (k c) -> p k c", k=4)
        nc.vector.tensor_mul(h_dst_3d, o3, th3)
        return c_new

    # ---- encoder -----------------------------------------------------
    hT_enc = None
    c = None
    for tt in range(LQ):
        emit_a_gather()
        if tt % 3 == 2:
            emit_a_gather()
        xTt = [qT[k][:, 32 * tt:32 * (tt + 1)] for k in range(2)]
        if hT_enc is None:
            h_ap = None
        else:
            hp = hT_enc
            h_ap = (lambda kc, hp=hp: hp[:, 32 * kc:32 * (kc + 1)])
        h_new = state.tile([128, 128], F16, tag="hT", name="hT")
        c = step(beT, xTt, wih_e, whh_e, h_ap, c,
                 h_new[:].rearrange("p (k c) -> p k c", k=4))
        hT_enc = h_new
        emit_a_transpose()
        emit_a_transpose()

    # ---- q_out^T and qb^T -------------------------------------------
    # qoT [256, 32] = q_lin_w @ h + q_lin_b, as 2 tiles of [128, 32]
    qo_ps = ps_p.tile([128, 512], F32, space="PSUM", tag="proj", name="qo_ps")
    for qt in range(2):
        sl = slice(32 * qt, 32 * (qt + 1))
        nc.tensor.matmul(qo_ps[:, sl], I128h[:], qlbT[:, sl],
                         start=True, stop=False, skip_group_check=True)
        for kc in range(4):
            nc.tensor.matmul(qo_ps[:, sl],
                             qlw(kc, slice(128 * qt, 128 * (qt + 1))),
                             hT_enc[:, 32 * kc:32 * (kc + 1)],
                             start=False, stop=(kc == 3),
                             skip_group_check=True)
    qoT = seqp.tile([128, 64], F16, tag="qoT", name="qoT")
    nc.scalar.activation(qoT[:], qo_ps[:, 0:64], AF.Identity)
    # qbT [128, 512] f16: (W_ihQ @ q_out + bias_dec) tiled [p, 32mt+b]
    qb_ps = ps_p.tile([128, 512], F32, space="PSUM", tag="proj", name="qb_ps")
    for mt in range(NMT):
        sl = slice(32 * mt, 32 * (mt + 1))
        msl = slice(128 * mt, 128 * (mt + 1))
        nc.tensor.matmul(qb_ps[:, sl], I128h[:], bdT[:, sl],
                         start=True, stop=False, skip_group_check=True)
        for kq in range(2):
            nc.tensor.matmul(qb_ps[:, sl], wihQ(kq, msl),
                             qoT[:, 32 * kq:32 * (kq + 1)],
                             start=False, stop=(kq == 1),
                             skip_group_check=True)
    qbT = seqp.tile([128, 512], F16, tag="qbT", name="qbT")
    nc.scalar.activation(qbT[:], qb_ps[:], AF.Identity)

    # ---- decoder + projection, software-pipelined --------------------
    out = t["out"]
    evac_n = 0

    def proj_vt(hs_prev, kb_prev, vt, stage):
        nonlocal evac_n
        pp = ps_p.tile([128, 512], F32, space="PSUM", tag="proj", name="pp")
        for kc in range(4):
            nc.tensor.matmul(pp[:], linw(kc, slice(128 * vt, 128 * (vt + 1))),
                             hs_prev[:, 512 * kc:512 * (kc + 1)],
                             start=(kc == 0), stop=(kc == 3),
                             skip_group_check=True)
        q = vt % 4
        dst = stage[:, 512 * q:512 * (q + 1)]
        if evac_n % 2 == 0:
            nc.scalar.activation(dst, pp[:], AF.Identity)
        else:
            nc.vector.tensor_copy(dst, pp[:])
        evac_n += 1
        if q == 3:
            ddst = out[128 * (vt - 3):128 * (vt + 1),
                       TOK * kb_prev:TOK * (kb_prev + 1)] \
                .rearrange("(v p) c -> p v c", v=4)
            nc.sync.dma_start(ddst, stage[:].rearrange("p (v c) -> p v c", v=4))

    hs_prev = None
    stage = None
    # h accessor: encoder final state, layout [u, 32*kc + b]
    h_acc = (lambda kc, hp=hT_enc: hp[:, 32 * kc:32 * (kc + 1)])
    for kb in range(NBLK):
        hs = hsp.tile([128, 4 * TOK], F16, tag="hs", name="hs")
        for st in range(BLK):
            xTt = [aT[k][:, 32 * (BLK * kb + st):32 * (BLK * kb + st) + 32]
                   for k in range(2)]
            hd3 = hs[:].rearrange("p (k c) -> p k c", k=4)[:, :, 32 * st:
                                                          32 * (st + 1)]
            c = step(qbT, xTt, wihA, whh_d, h_acc, c, hd3)
            # h(t) now lives at hs cols (512*kc + 32*st + b)
            h_acc = (lambda kc, hp=hs, s=st: hp[:, 512 * kc + 32 * s:
                                                512 * kc + 32 * s + 32])
            if hs_prev is not None:
                if st % 2 == 0:
                    stage = outp.tile([128, 2048], F16, tag="ostage",
                                      name="ostage")
                proj_vt(hs_prev, kb - 1, 2 * st, stage)
                proj_vt(hs_prev, kb - 1, 2 * st + 1, stage)
            else:
                emit_a_transpose()
                emit_a_transpose()
                emit_a_transpose()
        hs_prev = hs
    # tail: last block's projection
    for vt in range(NVT):
        if vt % 4 == 0:
            stage = outp.tile([128, 2048], F16, tag="ostage", name="ostage")
        proj_vt(hs_prev, NBLK - 1, vt, stage)


def kernel(**inputs):
    inp = {k: np.asarray(v) for k, v in inputs.items()}
    if "prog" not in _cache:
        _cache["prog"] = build_program()
    nc = _cache["prog"]

    perm = _gate_perm()
    f16 = np.float16

    def tile_bias(bias_perm, n):
        # [n*128] -> [128, n*32]: out[p, 32*mt + b] = bias[128*mt + p]
        return np.repeat(bias_perm.reshape(n, 128).T[:, :, None], 32,
                         axis=2).reshape(128, n * 32).astype(f16)

    def prep_lstm(w_ih, w_hh, b_ih, b_hh):
        wihT = np.ascontiguousarray(w_ih.T[:, perm]).astype(f16)
        whhT = np.ascontiguousarray(w_hh.T[:, perm]).astype(f16)
        bias = (b_ih + b_hh)[perm]
        return wihT, whhT, tile_bias(bias, NMT)

    wihT_e, whhT_e, beT = prep_lstm(inp["q_lstm_w_ih"], inp["q_lstm_w_hh"],
                                    inp["q_lstm_b_ih"], inp["q_lstm_b_hh"])
    wihT_d, whhT_d, bdT = prep_lstm(inp["a_lstm_w_ih"], inp["a_lstm_w_hh"],
                                    inp["a_lstm_b_ih"], inp["a_lstm_b_hh"])
    wihAT = np.ascontiguousarray(wihT_d[:EMB])
    wihQT = np.ascontiguousarray(wihT_d[EMB:])

    q_idx = np.zeros(13 * 128, np.int32)
    q_idx[:B * LQ] = inp["question"].T.reshape(-1).astype(np.int32)
    a_idx = inp["answer"][:, :STEPS].T.reshape(-1).astype(np.int32)

    lin_w = inp["lin_w"].astype(np.float32)   # [32000, 512]
    lin_b = inp["lin_b"].astype(np.float32)

    base = {
        "q_idx": q_idx, "a_idx": a_idx,
        "q_emb": inp["q_emb_w"].astype(f16),
        "a_emb": inp["a_emb_w"].astype(f16),
        "wihT_e": wihT_e, "whhT_e": whhT_e, "beT": beT,
        "wihAT": wihAT, "wihQT": wihQT, "whhT_d": whhT_d, "bdT": bdT,
        "qlwT": np.ascontiguousarray(inp["q_lin_w"].T).astype(f16),
        "qlbT": tile_bias(inp["q_lin_b"].astype(np.float64), 2),
        "i128h": np.eye(128, dtype=f16),
    }
    in_maps = []
    for core in range(NCORES):
        m = dict(base)
        sl = lin_w[VSH * core: VSH * (core + 1)]          # [4000, 512]
        slp = np.zeros((VPAD, HID), np.float32)
        slp[:VSH] = sl
        m["lin_wT"] = np.ascontiguousarray(slp.T).astype(f16)
        in_maps.append(m)

    _cache["in_maps"] = in_maps
    res = run_bass_kernel_spmd(nc, in_maps, core_ids=list(range(NCORES)))
    _cache["last_res"] = res

    final = np.empty((B, W_VOCAB, STEPS), np.float32)
    for core in range(NCORES):
        arr = np.asarray(res.results[core]["out"], dtype=np.float32)
        # [4096, 8192] col = 512*kb + 32*st + b; t = 16*kb + st
        arr = arr.reshape(VPAD, NBLK, BLK, B).transpose(3, 0, 1, 2) \
            .reshape(B, VPAD, STEPS)
        final[:, VSH * core:VSH * (core + 1), :] = arr[:, :VSH, :]
    final += lin_b[None, :, None]
    return final


if __name__ == "__main__":
    import reference
    ins = reference.setup_inputs()
    ref = np.asarray(reference.reference(**ins))
    got = kernel(**{k: np.asarray(v) for k, v in ins.items()})
    err = np.abs(got - ref).max() / (np.abs(ref).max() + 1e-12)
    print("max abs err:", np.abs(got - ref).max(), "rel:", err)


def run_traced():
    nc = _cache["prog"]
    return run_bass_kernel_spmd(nc, _cache["in_maps"],
                                core_ids=list(range(NCORES)), trace=True)


# revision 12
# speedup vs baseline: 1.9109x; 1.0023x over previous
"""Trainium2 Bass kernel for an LSTM encoder-decoder chatbot model.

Model: question -> embed -> LSTM(512) -> linear(256) = q_out
       answer[:, :256] -> embed -> concat(q_out) -> LSTM(512) -> linear(32000)
Output: logits [B=32, W=32000, STEPS=256] f32.

Sharding: all 8 cores run the full (replicated) encoder + decoder
recurrence; the 512x32000 output projection is sharded column-wise
(vocab) across cores; each core emits a [4096, 8192] f16 tile that the
host reshapes to [32, 4000, 256] and bias-adds.

Matmul strategy (cost model charges out-free-size N per instruction,
independent of K/M): gates are computed TRANSPOSED — gate units on the
128 PSUM partitions, batch (32) on the free dim. Each step is 16
sequential per-bank accumulation groups x 7 matmuls (1 bias/q seed via
identity, 2 x-chunks, 4 h-chunks) of N=32, i.e. 3584 PE rows/step vs
14336 for the batch-on-partition layout. h emerges already transposed
(no per-step PE transpose), written straight into a [128, 4*512] hs
block that feeds the vocab projection as the moving operand.
"""
import sys
import numpy as np

sys.path.insert(0, '/opt/trn_rl_repo')

import concourse.bass as bass  # noqa: E402
import concourse.bacc as bacc  # noqa: E402
import concourse.mybir as mybir  # noqa: E402
import concourse.tile as tile  # noqa: E402
from concourse.bass import IndirectOffsetOnAxis  # noqa: E402
from concourse.bass_utils import run_bass_kernel_spmd  # noqa: E402

F32 = mybir.dt.float32
F16 = mybir.dt.float16
I32DT = mybir.dt.int32
AF = mybir.ActivationFunctionType

W_VOCAB = 32000
EMB = 256
STEPS = 256
HID = 512
QOUT = 256
B = 32
LQ = 50
NCORES = 8
VSH = W_VOCAB // NCORES       # 4000 vocab rows per core
VPAD = 4096                   # padded to 32 tiles of 128
G = 4 * HID                   # 2048 gate units
NMT = G // 128                # 16 gate tiles per step
BLK = 16                      # decoder steps per hs block
NBLK = STEPS // BLK           # 16 blocks
TOK = BLK * B                 # 512 tokens per block
NVT = VPAD // 128             # 32 vocab tiles per core

_cache = {}


def _gate_perm():
    """Permuted gate unit g^ = 512*grp + 128*blk + u with grp order
    (i, f, o, g) <- old row 512*old_gate + 128*blk + u. Gate tile
    mt = g^//128 = 4*grp + blk, so PSUM cols [0:128)=i, [128:256)=f,
    [256:384)=o, [384:512)=g, each laid [p=u, 32*blk + b] — identical
    to the h/c layout [u, 32*blk + b]."""
    j = np.arange(G)
    grp, r = j // 512, j % 512
    blk, u = r // 128, r % 128
    old_gate = np.array([0, 1, 3, 2])[grp]
    return 512 * old_gate + 128 * blk + u


def build_program():
    nc = bacc.Bacc("TRN2", target_bir_lowering=False, debug=False,
                   num_devices=NCORES)

    def inp(name, shape, dt):
        return nc.dram_tensor(name, shape, dt, kind="ExternalInput").ap()

    q_idx = inp("q_idx", [13 * 128], I32DT)            # padded 1664, t-major
    a_idx = inp("a_idx", [64 * 128], I32DT)            # 8192, t-major
    q_emb = inp("q_emb", [W_VOCAB, EMB], F16)
    a_emb = inp("a_emb", [W_VOCAB, EMB], F16)
    wihT_e = inp("wihT_e", [EMB, G], F16)              # permuted cols
    whhT_e = inp("whhT_e", [HID, G], F16)
    beT = inp("beT", [128, 512], F16)                  # enc bias, tiled [p, 32mt+b]
    wihAT = inp("wihAT", [EMB, G], F16)
    wihQT = inp("wihQT", [QOUT, G], F16)
    whhT_d = inp("whhT_d", [HID, G], F16)
    bdT = inp("bdT", [128, 512], F16)                  # dec bias, tiled
    qlwT = inp("qlwT", [HID, QOUT], F16)
    qlbT = inp("qlbT", [128, 64], F16)                 # q_lin_b tiled [p, 32qt+b]
    lin_wT = inp("lin_wT", [HID, VPAD], F16)           # per-core slice
    i128h = inp("i128h", [128, 128], F16)
    out = nc.dram_tensor("out", [VPAD, NBLK * TOK], F16,
                         kind="ExternalOutput").ap()

    with tile.TileContext(nc) as tc:
        _build(nc, tc, locals())
    nc.compile()
    return nc


def _build(nc, tc, t):
    from contextlib import ExitStack
    ctx = ExitStack()
    with ctx:
        _build_inner(nc, tc, t, ctx)


def _build_inner(nc, tc, t, ctx):
    # ---- pools -------------------------------------------------------
    wpool = ctx.enter_context(tc.tile_pool(name="weights", bufs=1))
    const = ctx.enter_context(tc.tile_pool(name="const", bufs=1))
    embp = ctx.enter_context(tc.tile_pool(name="embp", bufs=4))
    seqp = ctx.enter_context(tc.tile_pool(name="seqp", bufs=1))
    state = ctx.enter_context(tc.tile_pool(name="state", bufs=2))
    ew = ctx.enter_context(tc.tile_pool(name="ew", bufs=2))
    hsp = ctx.enter_context(tc.tile_pool(name="hsp", bufs=3))
    outp = ctx.enter_context(tc.tile_pool(name="outp", bufs=3))
    ps_g = ctx.enter_context(tc.tile_pool(name="ps_g", bufs=2, space="PSUM"))
    ps_tr = ctx.enter_context(tc.tile_pool(name="ps_tr", bufs=1, space="PSUM"))
    ps_p = ctx.enter_context(tc.tile_pool(name="ps_p", bufs=3, space="PSUM"))

    def load(pool, ap, dt=None, name=None):
        s = pool.tile(list(ap.shape), dt or ap.dtype, tag=name, name=name or 'ld')
        nc.sync.dma_start(s[:], ap[:])
        return s

    def loadc(pool, ap, name):
        p, cdim = ap.shape
        n = p // 128
        s = pool.tile([128, n * cdim], ap.dtype, tag=name, name=name)
        for k in range(n):
            nc.sync.dma_start(s[:, cdim * k:cdim * (k + 1)],
                              ap[128 * k:128 * (k + 1), :])

        def chunk(k, sl=slice(None)):
            base = cdim * k
            if sl == slice(None):
                return s[:, base:base + cdim]
            return s[:, base + sl.start:base + sl.stop]
        return chunk

    # ---- resident weights/constants ---------------------------------
    # encoder-critical loads first so the q gathers aren't stuck behind
    # the big projection/decoder weight DMAs on the shared DMA engines
    I128h = load(const, t["i128h"], name="I128h")
    qidx_sb = load(const, t["q_idx"].rearrange("(n p) -> n p", p=128)
                   .rearrange("n p -> p n"), name="qidx")   # [128, 13]
    wih_e = loadc(wpool, t["wihT_e"], "wih_e")       # 2 chunks [128, 2048]
    whh_e = loadc(wpool, t["whhT_e"], "whh_e")       # 4 chunks
    beT = load(const, t["beT"], name="beT")

    # ---- embedding gather + transpose helpers ------------------------
    # xT tiles: [128 = emb unit within chunk, token] f16, token t-major.
    qT = [seqp.tile([128, 13 * 128], F16, tag=f"qT{k}", name=f"qT{k}")
          for k in range(2)]
    aT = [seqp.tile([128, 64 * 128], F16, tag=f"aT{k}", name=f"aT{k}")
          for k in range(2)]

    def gather(table, idx_sb, i, name):
        rows = embp.tile([128, EMB], F16, tag="gather", name=name)
        nc.gpsimd.indirect_dma_start(
            out=rows[:], out_offset=None, in_=table[:],
            in_offset=IndirectOffsetOnAxis(ap=idx_sb[:, i:i + 1], axis=0))
        return rows

    def transpose_into(xT, rows, i, k):
        p = ps_tr.tile([128, 128], F16, space="PSUM", tag="tr", name="trp")
        nc.tensor.transpose(p[:], rows[:, 128 * k:128 * (k + 1)], I128h[:])
        nc.vector.tensor_copy(xT[k][:, 128 * i:128 * (i + 1)], p[:])

    # q side fully upfront (needed from encoder step 0; also ramps PE)
    for i in range(13):
        rows = gather(t["q_emb"], qidx_sb, i, "qrows")
        for k in range(2):
            transpose_into(qT, rows, i, k)

    # remaining (decoder/projection) weights — DMA overlaps the encoder
    aidx_sb = load(const, t["a_idx"].rearrange("(n p) -> n p", p=128)
                   .rearrange("n p -> p n"), name="aidx")   # [128, 64]
    wihA = loadc(wpool, t["wihAT"], "wihA")
    wihQ = loadc(wpool, t["wihQT"], "wihQ")
    whh_d = loadc(wpool, t["whhT_d"], "whh_d")
    bdT = load(const, t["bdT"], name="bdT")
    qlw = loadc(wpool, t["qlwT"], "qlw")             # 4 chunks [128, 256]
    qlbT = load(const, t["qlbT"], name="qlbT")
    linw = loadc(wpool, t["lin_wT"], "linw")         # 4 chunks [128, 4096]

    # a side: gathers + transposes interleaved into encoder/early decoder
    a_rows = [None] * 64
    a_gathered = 0
    a_transposed = 0

    def emit_a_gather():
        nonlocal a_gathered
        if a_gathered < 64:
            a_rows[a_gathered] = gather(t["a_emb"], aidx_sb, a_gathered,
                                        "arows")
            a_gathered += 1

    def emit_a_transpose():
        nonlocal a_transposed
        if a_transposed < 2 * 64 and a_transposed < 2 * a_gathered:
            i, k = divmod(a_transposed, 2)
            transpose_into(aT, a_rows[i], i, k)
            a_transposed += 1

    # ---- one LSTM step, transposed-gates layout ----------------------
    # Returns (c_new, h_getter) where the new h^T lives either in a state
    # tile (encoder) or inside the current hs block (decoder).
    def step(seed_sb, xTt, wih, whh, h_ap, c_prev, h_dst_3d):
        """seed_sb: [128, 512] f16 SBUF (bias [+ q proj]), seeded via I128.
        xTt: per-chunk rhs aps ([128, 32] each) for the 2 x chunks.
        h_ap(kc): rhs ap [128, 32] for h chunk kc, or None (t=0 encoder).
        c_prev: [128, 128] f32 SBUF tile or None.
        h_dst_3d: [p, 4, 32] view where the DVE writes h^T.
        """
        # two half-banks so the cell can start after the i,f tiles:
        # bank A cols = i (0:128) | f (128:256); bank B = g (0:128) | o
        # (128:256). PSUM column of permuted gate tile mt:
        #   mt 0..7  (i,f) -> A[:, 32*mt]
        #   mt 12..15 (g)  -> B[:, 32*(mt-12)]
        #   mt 8..11  (o)  -> B[:, 128 + 32*(mt-8)]
        gA = ps_g.tile([128, 256], F32, space="PSUM", tag="gA", name="gA")
        gB = ps_g.tile([128, 256], F32, space="PSUM", tag="gB", name="gB")
        has_h = h_ap is not None

        def emit_tile(gp, col, mt):
            sl = slice(col, col + 32)
            msl = slice(128 * mt, 128 * (mt + 1))
            n_mm = 3 + (4 if has_h else 0)
            i_mm = 0
            nc.tensor.matmul(gp[:, sl], I128h[:], seed_sb[:, 32 * mt:
                                                          32 * mt + 32],
                             start=True, stop=(n_mm == 1),
                             skip_group_check=True)
            i_mm += 1
            for kx in range(2):
                nc.tensor.matmul(gp[:, sl], wih(kx, msl), xTt[kx],
                                 start=False, stop=(i_mm == n_mm - 1),
                                 skip_group_check=True)
                i_mm += 1
            if has_h:
                for kc in range(4):
                    nc.tensor.matmul(gp[:, sl], whh(kc, msl), h_ap(kc),
                                     start=False, stop=(i_mm == n_mm - 1),
                                     skip_group_check=True)
                    i_mm += 1

        for mt in range(8):                      # i, f
            emit_tile(gA, 32 * mt, mt)
        for mt in range(12, 16):                 # g first in bank B
            emit_tile(gB, 32 * (mt - 12), mt)
        for mt in range(8, 12):                  # o last
            emit_tile(gB, 128 + 32 * (mt - 8), mt)

        sif = ew.tile([128, 256], F16, tag="sif", name="sif")
        nc.scalar.activation(sif[:], gA[:], AF.Sigmoid)
        gg = ew.tile([128, 128], F16, tag="gg", name="gg")
        nc.scalar.activation(gg[:], gB[:, 0:128], AF.Tanh)
        so = ew.tile([128, 128], F16, tag="so", name="so")
        nc.scalar.activation(so[:], gB[:, 128:256], AF.Sigmoid)
        c_new = state.tile([128, 128], F32, tag="c", name="c")
        if c_prev is None:
            nc.vector.tensor_mul(c_new[:], sif[:, 0:128], gg[:])
        else:
            fc = ew.tile([128, 128], F32, tag="fc", name="fc")
            nc.gpsimd.tensor_mul(fc[:], sif[:, 128:256], c_prev[:])
            igg = ew.tile([128, 128], F16, tag="igg", name="igg")
            nc.vector.tensor_mul(igg[:], sif[:, 0:128], gg[:])
            nc.vector.tensor_add(c_new[:], igg[:], fc[:])
        th = ew.tile([128, 128], F16, tag="th", name="th")
        nc.scalar.activation(th[:], c_new[:], AF.Tanh)
        o3 = so[:].rearrange("p (k c) -> p k c", k=4)
        th3 = th[:].rearrange("p (k c) -> p k c", k=4)
        nc.vector.tensor_mul(h_dst_3d, o3, th3)
        return c_new

    # ---- encoder -----------------------------------------------------
    hT_enc = None
    c = None
    for tt in range(LQ):
        emit_a_gather()
        if tt % 3 == 2:
            emit_a_gather()
        xTt = [qT[k][:, 32 * tt:32 * (tt + 1)] for k in range(2)]
        if hT_enc is None:
            h_ap = None
        else:
            hp = hT_enc
            h_ap = (lambda kc, hp=hp: hp[:, 32 * kc:32 * (kc + 1)])
        h_new = state.tile([128, 128], F16, tag="hT", name="hT")
        c = step(beT, xTt, wih_e, whh_e, h_ap, c,
                 h_new[:].rearrange("p (k c) -> p k c", k=4))
        hT_enc = h_new
        emit_a_transpose()
        emit_a_transpose()

    # ---- q_out^T and qb^T -------------------------------------------
    # qoT [256, 32] = q_lin_w @ h + q_lin_b, as 2 tiles of [128, 32]
    qo_ps = ps_p.tile([128, 512], F32, space="PSUM", tag="proj", name="qo_ps")
    for qt in range(2):
        sl = slice(32 * qt, 32 * (qt + 1))
        nc.tensor.matmul(qo_ps[:, sl], I128h[:], qlbT[:, sl],
                         start=True, stop=False, skip_group_check=True)
        for kc in range(4):
            nc.tensor.matmul(qo_ps[:, sl],
                             qlw(kc, slice(128 * qt, 128 * (qt + 1))),
                             hT_enc[:, 32 * kc:32 * (kc + 1)],
                             start=False, stop=(kc == 3),
                             skip_group_check=True)
    qoT = seqp.tile([128, 64], F16, tag="qoT", name="qoT")
    nc.scalar.activation(qoT[:], qo_ps[:, 0:64], AF.Identity)
    # qbT [128, 512] f16: (W_ihQ @ q_out + bias_dec) tiled [p, 32mt+b]
    qb_ps = ps_p.tile([128, 512], F32, space="PSUM", tag="proj", name="qb_ps")
    for mt in range(NMT):
        sl = slice(32 * mt, 32 * (mt + 1))
        msl = slice(128 * mt, 128 * (mt + 1))
        nc.tensor.matmul(qb_ps[:, sl], I128h[:], bdT[:, sl],
                         start=True, stop=False, skip_group_check=True)
        for kq in range(2):
            nc.tensor.matmul(qb_ps[:, sl], wihQ(kq, msl),
                             qoT[:, 32 * kq:32 * (kq + 1)],
                             start=False, stop=(kq == 1),
                             skip_group_check=True)
    qbT = seqp.tile([128, 512], F16, tag="qbT", name="qbT")
    nc.scalar.activation(qbT[:], qb_ps[:], AF.Identity)

    # ---- decoder + projection, software-pipelined --------------------
    out = t["out"]
    evac_n = 0
    pending = []        # (pp, kb_prev, vt) awaiting evac+dma
    stage_ref = [None]

    def proj_mm(hs_prev, kb_prev, vt):
        pp = ps_p.tile([128, 512], F32, space="PSUM", tag="proj", name="pp")
        for kc in range(4):
            nc.tensor.matmul(pp[:], linw(kc, slice(128 * vt, 128 * (vt + 1))),
                             hs_prev[:, 512 * kc:512 * (kc + 1)],
                             start=(kc == 0), stop=(kc == 3),
                             skip_group_check=True)
        pending.append((pp, kb_prev, vt))

    def flush_pending():
        nonlocal evac_n
        for pp, kb_prev, vt in pending:
            q = vt % 4
            if q == 0:
                stage_ref[0] = outp.tile([128, 2048], F16, tag="ostage",
                                         name="ostage")
            stage = stage_ref[0]
            dst = stage[:, 512 * q:512 * (q + 1)]
            if evac_n % 2 == 0:
                nc.scalar.activation(dst, pp[:], AF.Identity)
            else:
                nc.vector.tensor_copy(dst, pp[:])
            evac_n += 1
            if q == 3:
                ddst = out[128 * (vt - 3):128 * (vt + 1),
                           TOK * kb_prev:TOK * (kb_prev + 1)] \
                    .rearrange("(v p) c -> p v c", v=4)
                nc.sync.dma_start(ddst,
                                  stage[:].rearrange("p (v c) -> p v c", v=4))
        pending.clear()

    hs_prev = None
    # h accessor: encoder final state, layout [u, 32*kc + b]
    h_acc = (lambda kc, hp=hT_enc: hp[:, 32 * kc:32 * (kc + 1)])
    for kb in range(NBLK):
        hs = hsp.tile([128, 4 * TOK], F16, tag="hs", name="hs")
        for st in range(BLK):
            flush_pending()    # prev step's evac+dma land BEFORE cell ops
            xTt = [aT[k][:, 32 * (BLK * kb + st):32 * (BLK * kb + st) + 32]
                   for k in range(2)]
            hd3 = hs[:].rearrange("p (k c) -> p k c", k=4)[:, :, 32 * st:
                                                          32 * (st + 1)]
            c = step(qbT, xTt, wihA, whh_d, h_acc, c, hd3)
            # h(t) now lives at hs cols (512*kc + 32*st + b)
            h_acc = (lambda kc, hp=hs, s=st: hp[:, 512 * kc + 32 * s:
                                                512 * kc + 32 * s + 32])
            if hs_prev is not None:
                proj_mm(hs_prev, kb - 1, 2 * st)
                proj_mm(hs_prev, kb - 1, 2 * st + 1)
            else:
                emit_a_transpose()
                emit_a_transpose()
                emit_a_transpose()
        hs_prev = hs
    # tail: last block's projection
    for vt in range(NVT):
        proj_mm(hs_prev, NBLK - 1, vt)
        if vt % 2 == 1:
            flush_pending()
    flush_pending()


def kernel(**inputs):
    inp = {k: np.asarray(v) for k, v in inputs.items()}
    if "prog" not in _cache:
        _cache["prog"] = build_program()
    nc = _cache["prog"]

    perm = _gate_perm()
    f16 = np.float16

    def tile_bias(bias_perm, n):
        # [n*128] -> [128, n*32]: out[p, 32*mt + b] = bias[128*mt + p]
        return np.repeat(bias_perm.reshape(n, 128).T[:, :, None], 32,
                         axis=2).reshape(128, n * 32).astype(f16)

    def prep_lstm(w_ih, w_hh, b_ih, b_hh):
        wihT = np.ascontiguousarray(w_ih.T[:, perm]).astype(f16)
        whhT = np.ascontiguousarray(w_hh.T[:, perm]).astype(f16)
        bias = (b_ih + b_hh)[perm]
        return wihT, whhT, tile_bias(bias, NMT)

    wihT_e, whhT_e, beT = prep_lstm(inp["q_lstm_w_ih"], inp["q_lstm_w_hh"],
                                    inp["q_lstm_b_ih"], inp["q_lstm_b_hh"])
    wihT_d, whhT_d, bdT = prep_lstm(inp["a_lstm_w_ih"], inp["a_lstm_w_hh"],
                                    inp["a_lstm_b_ih"], inp["a_lstm_b_hh"])
    wihAT = np.ascontiguousarray(wihT_d[:EMB])
    wihQT = np.ascontiguousarray(wihT_d[EMB:])

    q_idx = np.zeros(13 * 128, np.int32)
    q_idx[:B * LQ] = inp["question"].T.reshape(-1).astype(np.int32)
    a_idx = inp["answer"][:, :STEPS].T.reshape(-1).astype(np.int32)

    lin_w = inp["lin_w"].astype(np.float32)   # [32000, 512]
    lin_b = inp["lin_b"].astype(np.float32)

    base = {
        "q_idx": q_idx, "a_idx": a_idx,
        "q_emb": inp["q_emb_w"].astype(f16),
        "a_emb": inp["a_emb_w"].astype(f16),
        "wihT_e": wihT_e, "whhT_e": whhT_e, "beT": beT,
        "wihAT": wihAT, "wihQT": wihQT, "whhT_d": whhT_d, "bdT": bdT,
        "qlwT": np.ascontiguousarray(inp["q_lin_w"].T).astype(f16),
        "qlbT": tile_bias(inp["q_lin_b"].astype(np.float64), 2),
        "i128h": np.eye(128, dtype=f16),
    }
    in_maps = []
    for core in range(NCORES):
        m = dict(base)
        sl = lin_w[VSH * core: VSH * (core + 1)]          # [4000, 512]
        slp = np.zeros((VPAD, HID), np.float32)
        slp[:VSH] = sl
        m["lin_wT"] = np.ascontiguousarray(slp.T).astype(f16)
        in_maps.append(m)

    _cache["in_maps"] = in_maps
    res = run_bass_kernel_spmd(nc, in_maps, core_ids=list(range(NCORES)))
    _cache["last_res"] = res

    final = np.empty((B, W_VOCAB, STEPS), np.float32)
    for core in range(NCORES):
        arr = np.asarray(res.results[core]["out"], dtype=np.float32)
        # [4096, 8192] col = 512*kb + 32*st + b; t = 16*kb + st
        arr = arr.reshape(VPAD, NBLK, BLK, B).transpose(3, 0, 1, 2) \
            .reshape(B, VPAD, STEPS)
        final[:, VSH * core:VSH * (core + 1), :] = arr[:, :VSH, :]
    final += lin_b[None, :, None]
    return final


if __name__ == "__main__":
    import reference
    ins = reference.setup_inputs()
    ref = np.asarray(reference.reference(**ins))
    got = kernel(**{k: np.asarray(v) for k, v in ins.items()})
    err = np.abs(got - ref).max() / (np.abs(ref).max() + 1e-12)
    print("max abs err:", np.abs(got - ref).max(), "rel:", err)


def run_traced():
    nc = _cache["prog"]
    return run_bass_kernel_spmd(nc, _cache["in_maps"],
                                core_ids=list(range(NCORES)), trace=True)


# revision 13
# speedup vs baseline: 2.0210x; 1.0577x over previous
"""Trainium2 Bass kernel for an LSTM encoder-decoder chatbot model.

Model: question -> embed -> LSTM(512) -> linear(256) = q_out
       answer[:, :256] -> embed -> concat(q_out) -> LSTM(512) -> linear(32000)
Output: logits [B=32, W=32000, STEPS=256] f32.

Sharding: all 8 cores run the full (replicated) encoder + decoder
recurrence; the 512x32000 output projection is sharded column-wise
(vocab) across cores; each core emits a [4096, 8192] f16 tile that the
host reshapes to [32, 4000, 256] and bias-adds.

Matmul strategy (cost model charges out-free-size N per instruction,
independent of K/M): gates are computed TRANSPOSED — gate units on the
128 PSUM partitions, batch (32) on the free dim. Each step is 16
sequential per-bank accumulation groups x 7 matmuls (1 bias/q seed via
identity, 2 x-chunks, 4 h-chunks) of N=32, i.e. 3584 PE rows/step vs
14336 for the batch-on-partition layout. h emerges already transposed
(no per-step PE transpose), written straight into a [128, 4*512] hs
block that feeds the vocab projection as the moving operand.
"""
import sys
import numpy as np

sys.path.insert(0, '/opt/trn_rl_repo')

import concourse.bass as bass  # noqa: E402
import concourse.bacc as bacc  # noqa: E402
import concourse.mybir as mybir  # noqa: E402
import concourse.tile as tile  # noqa: E402
from concourse.bass import IndirectOffsetOnAxis  # noqa: E402
from concourse.bass_utils import run_bass_kernel_spmd  # noqa: E402

F32 = mybir.dt.float32
F16 = mybir.dt.float16
I32DT = mybir.dt.int32
AF = mybir.ActivationFunctionType

W_VOCAB = 32000
EMB = 256
STEPS = 256
HID = 512
QOUT = 256
B = 32
LQ = 50
NCORES = 8
VSH = W_VOCAB // NCORES       # 4000 vocab rows per core
VPAD = 4096                   # padded to 32 tiles of 128
G = 4 * HID                   # 2048 gate units
NMT = G // 128                # 16 gate tiles per step
BLK = 16                      # decoder steps per hs block
NBLK = STEPS // BLK           # 16 blocks
TOK = BLK * B                 # 512 tokens per block
NVT = VPAD // 128             # 32 vocab tiles per core

_cache = {}


def _gate_perm():
    """Permuted gate unit g^ = 512*grp + 128*blk + u with grp order
    (i, f, o, g) <- old row 512*old_gate + 128*blk + u. Gate tile
    mt = g^//128 = 4*grp + blk, so PSUM cols [0:128)=i, [128:256)=f,
    [256:384)=o, [384:512)=g, each laid [p=u, 32*blk + b] — identical
    to the h/c layout [u, 32*blk + b]."""
    j = np.arange(G)
    grp, r = j // 512, j % 512
    blk, u = r // 128, r % 128
    old_gate = np.array([0, 1, 3, 2])[grp]
    return 512 * old_gate + 128 * blk + u


def build_program():
    nc = bacc.Bacc("TRN2", target_bir_lowering=False, debug=False,
                   num_devices=NCORES)

    def inp(name, shape, dt):
        return nc.dram_tensor(name, shape, dt, kind="ExternalInput").ap()

    q_idx = inp("q_idx", [13 * 128], I32DT)            # padded 1664, t-major
    a_idx = inp("a_idx", [64 * 128], I32DT)            # 8192, t-major
    q_emb = inp("q_emb", [W_VOCAB, EMB], F16)
    a_emb = inp("a_emb", [W_VOCAB, EMB], F16)
    wihT_e = inp("wihT_e", [EMB, G], F16)              # permuted cols
    whhT_e = inp("whhT_e", [HID, G], F16)
    beT = inp("beT", [128, 512], F16)                  # enc bias, tiled [p, 32mt+b]
    wihAT = inp("wihAT", [EMB, G], F16)
    wihQT = inp("wihQT", [QOUT, G], F16)
    whhT_d = inp("whhT_d", [HID, G], F16)
    bdT = inp("bdT", [128, 512], F16)                  # dec bias, tiled
    qlwT = inp("qlwT", [HID, QOUT], F16)
    qlbT = inp("qlbT", [128, 64], F16)                 # q_lin_b tiled [p, 32qt+b]
    lin_wT = inp("lin_wT", [HID, VPAD], F16)           # per-core slice
    i128h = inp("i128h", [128, 128], F16)
    out = nc.dram_tensor("out", [VPAD, NBLK * TOK], F16,
                         kind="ExternalOutput").ap()

    with tile.TileContext(nc) as tc:
        _build(nc, tc, locals())
    nc.compile()
    return nc


def _build(nc, tc, t):
    from contextlib import ExitStack
    ctx = ExitStack()
    with ctx:
        _build_inner(nc, tc, t, ctx)


def _build_inner(nc, tc, t, ctx):
    # ---- pools -------------------------------------------------------
    wpool = ctx.enter_context(tc.tile_pool(name="weights", bufs=1))
    const = ctx.enter_context(tc.tile_pool(name="const", bufs=1))
    embp = ctx.enter_context(tc.tile_pool(name="embp", bufs=4))
    seqp = ctx.enter_context(tc.tile_pool(name="seqp", bufs=1))
    state = ctx.enter_context(tc.tile_pool(name="state", bufs=2))
    ew = ctx.enter_context(tc.tile_pool(name="ew", bufs=2))
    hsp = ctx.enter_context(tc.tile_pool(name="hsp", bufs=3))
    outp = ctx.enter_context(tc.tile_pool(name="outp", bufs=3))
    ps_g = ctx.enter_context(tc.tile_pool(name="ps_g", bufs=2, space="PSUM"))
    ps_tr = ctx.enter_context(tc.tile_pool(name="ps_tr", bufs=1, space="PSUM"))
    ps_p = ctx.enter_context(tc.tile_pool(name="ps_p", bufs=3, space="PSUM"))

    def load(pool, ap, dt=None, name=None):
        s = pool.tile(list(ap.shape), dt or ap.dtype, tag=name, name=name or 'ld')
        nc.sync.dma_start(s[:], ap[:])
        return s

    def loadc(pool, ap, name):
        p, cdim = ap.shape
        n = p // 128
        s = pool.tile([128, n * cdim], ap.dtype, tag=name, name=name)
        for k in range(n):
            nc.sync.dma_start(s[:, cdim * k:cdim * (k + 1)],
                              ap[128 * k:128 * (k + 1), :])

        def chunk(k, sl=slice(None)):
            base = cdim * k
            if sl == slice(None):
                return s[:, base:base + cdim]
            return s[:, base + sl.start:base + sl.stop]
        return chunk

    # ---- resident weights/constants ---------------------------------
    # encoder-critical loads first so the q gathers aren't stuck behind
    # the big projection/decoder weight DMAs on the shared DMA engines
    I128h = load(const, t["i128h"], name="I128h")
    qidx_sb = load(const, t["q_idx"].rearrange("(n p) -> n p", p=128)
                   .rearrange("n p -> p n"), name="qidx")   # [128, 13]
    wih_e = loadc(wpool, t["wihT_e"], "wih_e")       # 2 chunks [128, 2048]
    whh_e = loadc(wpool, t["whhT_e"], "whh_e")       # 4 chunks
    beT = load(const, t["beT"], name="beT")

    # ---- embedding gather + transpose helpers ------------------------
    # xT tiles: [128 = emb unit within chunk, token] f16, token t-major.
    qT = [seqp.tile([128, 13 * 128], F16, tag=f"qT{k}", name=f"qT{k}")
          for k in range(2)]
    aT = [seqp.tile([128, 64 * 128], F16, tag=f"aT{k}", name=f"aT{k}")
          for k in range(2)]

    def gather(table, idx_sb, i, name):
        rows = embp.tile([128, EMB], F16, tag="gather", name=name)
        nc.gpsimd.indirect_dma_start(
            out=rows[:], out_offset=None, in_=table[:],
            in_offset=IndirectOffsetOnAxis(ap=idx_sb[:, i:i + 1], axis=0))
        return rows

    def transpose_into(xT, rows, i, k):
        p = ps_tr.tile([128, 128], F16, space="PSUM", tag="tr", name="trp")
        nc.tensor.transpose(p[:], rows[:, 128 * k:128 * (k + 1)], I128h[:])
        nc.vector.tensor_copy(xT[k][:, 128 * i:128 * (i + 1)], p[:])

    # q side fully upfront (needed from encoder step 0; also ramps PE)
    for i in range(13):
        rows = gather(t["q_emb"], qidx_sb, i, "qrows")
        for k in range(2):
            transpose_into(qT, rows, i, k)

    # remaining (decoder/projection) weights — DMA overlaps the encoder
    aidx_sb = load(const, t["a_idx"].rearrange("(n p) -> n p", p=128)
                   .rearrange("n p -> p n"), name="aidx")   # [128, 64]
    wihA = loadc(wpool, t["wihAT"], "wihA")
    wihQ = loadc(wpool, t["wihQT"], "wihQ")
    whh_d = loadc(wpool, t["whhT_d"], "whh_d")
    bdT = load(const, t["bdT"], name="bdT")
    qlw = loadc(wpool, t["qlwT"], "qlw")             # 4 chunks [128, 256]
    qlbT = load(const, t["qlbT"], name="qlbT")
    linw = loadc(wpool, t["lin_wT"], "linw")         # 4 chunks [128, 4096]

    # a side: gathers + transposes interleaved into encoder/early decoder
    a_rows = [None] * 64
    a_gathered = 0
    a_transposed = 0

    def emit_a_gather():
        nonlocal a_gathered
        if a_gathered < 64:
            a_rows[a_gathered] = gather(t["a_emb"], aidx_sb, a_gathered,
                                        "arows")
            a_gathered += 1

    def emit_a_transpose():
        nonlocal a_transposed
        if a_transposed < 2 * 64 and a_transposed < 2 * a_gathered:
            i, k = divmod(a_transposed, 2)
            transpose_into(aT, a_rows[i], i, k)
            a_transposed += 1

    # ---- one LSTM step, transposed-gates layout ----------------------
    # Returns (c_new, h_getter) where the new h^T lives either in a state
    # tile (encoder) or inside the current hs block (decoder).
    def step(seed_sb, xTt, wih, whh, h_ap, c_prev, h_dst_3d):
        """seed_sb: [128, 512] f16 SBUF (bias [+ q proj]), seeded via I128.
        xTt: per-chunk rhs aps ([128, 32] each) for the 2 x chunks.
        h_ap(kc): rhs ap [128, 32] for h chunk kc, or None (t=0 encoder).
        c_prev: [128, 128] f32 SBUF tile or None.
        h_dst_3d: [p, 4, 32] view where the DVE writes h^T.
        """
        # two half-banks so the cell can start after the i,f tiles:
        # bank A cols = i (0:128) | f (128:256); bank B = g (0:128) | o
        # (128:256). PSUM column of permuted gate tile mt:
        #   mt 0..7  (i,f) -> A[:, 32*mt]
        #   mt 12..15 (g)  -> B[:, 32*(mt-12)]
        #   mt 8..11  (o)  -> B[:, 128 + 32*(mt-8)]
        gA = ps_g.tile([128, 256], F32, space="PSUM", tag="gA", name="gA")
        gB = ps_g.tile([128, 256], F32, space="PSUM", tag="gB", name="gB")
        has_h = h_ap is not None

        def emit_tile(gp, col, mt):
            sl = slice(col, col + 32)
            msl = slice(128 * mt, 128 * (mt + 1))
            n_mm = 3 + (4 if has_h else 0)
            i_mm = 0
            nc.tensor.matmul(gp[:, sl], I128h[:], seed_sb[:, 32 * mt:
                                                          32 * mt + 32],
                             start=True, stop=(n_mm == 1),
                             skip_group_check=True)
            i_mm += 1
            for kx in range(2):
                nc.tensor.matmul(gp[:, sl], wih(kx, msl), xTt[kx],
                                 start=False, stop=(i_mm == n_mm - 1),
                                 skip_group_check=True)
                i_mm += 1
            if has_h:
                for kc in range(4):
                    nc.tensor.matmul(gp[:, sl], whh(kc, msl), h_ap(kc),
                                     start=False, stop=(i_mm == n_mm - 1),
                                     skip_group_check=True)
                    i_mm += 1

        for mt in range(8):                      # i, f
            emit_tile(gA, 32 * mt, mt)
        for mt in range(12, 16):                 # g first in bank B
            emit_tile(gB, 32 * (mt - 12), mt)
        for mt in range(8, 12):                  # o last
            emit_tile(gB, 128 + 32 * (mt - 8), mt)

        sif = ew.tile([128, 256], F16, tag="sif", name="sif")
        nc.scalar.activation(sif[:], gA[:], AF.Sigmoid)
        gg = ew.tile([128, 128], F16, tag="gg", name="gg")
        nc.scalar.activation(gg[:], gB[:, 0:128], AF.Tanh)
        so = ew.tile([128, 128], F16, tag="so", name="so")
        nc.scalar.activation(so[:], gB[:, 128:256], AF.Sigmoid)
        c_new = state.tile([128, 128], F32, tag="c", name="c")
        if c_prev is None:
            nc.vector.tensor_mul(c_new[:], sif[:, 0:128], gg[:])
        else:
            fc = ew.tile([128, 128], F32, tag="fc", name="fc")
            nc.gpsimd.tensor_mul(fc[:], sif[:, 128:256], c_prev[:])
            igg = ew.tile([128, 128], F16, tag="igg", name="igg")
            nc.vector.tensor_mul(igg[:], sif[:, 0:128], gg[:])
            nc.vector.tensor_add(c_new[:], igg[:], fc[:])
        th = ew.tile([128, 128], F16, tag="th", name="th")
        nc.scalar.activation(th[:], c_new[:], AF.Tanh)
        o3 = so[:].rearrange("p (k c) -> p k c", k=4)
        th3 = th[:].rearrange("p (k c) -> p k c", k=4)
        nc.vector.tensor_mul(h_dst_3d, o3, th3)
        return c_new

    # ---- encoder -----------------------------------------------------
    hT_enc = None
    c = None
    for tt in range(LQ):
        emit_a_gather()
        if tt % 3 == 2:
            emit_a_gather()
        xTt = [qT[k][:, 32 * tt:32 * (tt + 1)] for k in range(2)]
        if hT_enc is None:
            h_ap = None
        else:
            hp = hT_enc
            h_ap = (lambda kc, hp=hp: hp[:, 32 * kc:32 * (kc + 1)])
        h_new = state.tile([128, 128], F16, tag="hT", name="hT")
        c = step(beT, xTt, wih_e, whh_e, h_ap, c,
                 h_new[:].rearrange("p (k c) -> p k c", k=4))
        hT_enc = h_new
        emit_a_transpose()
        emit_a_transpose()

    # ---- q_out^T and qb^T -------------------------------------------
    # qoT [256, 32] = q_lin_w @ h + q_lin_b, as 2 tiles of [128, 32]
    qo_ps = ps_p.tile([128, 512], F32, space="PSUM", tag="proj", name="qo_ps")
    for qt in range(2):
        sl = slice(32 * qt, 32 * (qt + 1))
        nc.tensor.matmul(qo_ps[:, sl], I128h[:], qlbT[:, sl],
                         start=True, stop=False, skip_group_check=True)
        for kc in range(4):
            nc.tensor.matmul(qo_ps[:, sl],
                             qlw(kc, slice(128 * qt, 128 * (qt + 1))),
                             hT_enc[:, 32 * kc:32 * (kc + 1)],
                             start=False, stop=(kc == 3),
                             skip_group_check=True)
    qoT = seqp.tile([128, 64], F16, tag="qoT", name="qoT")
    nc.scalar.activation(qoT[:], qo_ps[:, 0:64], AF.Identity)
    # qbT [128, 512] f16: (W_ihQ @ q_out + bias_dec) tiled [p, 32mt+b]
    qb_ps = ps_p.tile([128, 512], F32, space="PSUM", tag="proj", name="qb_ps")
    for mt in range(NMT):
        sl = slice(32 * mt, 32 * (mt + 1))
        msl = slice(128 * mt, 128 * (mt + 1))
        nc.tensor.matmul(qb_ps[:, sl], I128h[:], bdT[:, sl],
                         start=True, stop=False, skip_group_check=True)
        for kq in range(2):
            nc.tensor.matmul(qb_ps[:, sl], wihQ(kq, msl),
                             qoT[:, 32 * kq:32 * (kq + 1)],
                             start=False, stop=(kq == 1),
                             skip_group_check=True)
    qbT = seqp.tile([128, 512], F16, tag="qbT", name="qbT")
    nc.scalar.activation(qbT[:], qb_ps[:], AF.Identity)

    # ---- decoder + projection, software-pipelined --------------------
    out = t["out"]
    evac_n = 0
    pending = []        # (pp, kb_prev, vt) awaiting evac+dma
    stage_ref = [None]

    def proj_mm(hs_prev, kb_prev, vt):
        pp = ps_p.tile([128, 512], F32, space="PSUM", tag="proj", name="pp")
        for kc in range(4):
            nc.tensor.matmul(pp[:], linw(kc, slice(128 * vt, 128 * (vt + 1))),
                             hs_prev[:, 512 * kc:512 * (kc + 1)],
                             start=(kc == 0), stop=(kc == 3),
                             skip_group_check=True)
        pending.append((pp, kb_prev, vt))

    def flush_pending():
        nonlocal evac_n
        for pp, kb_prev, vt in pending:
            q = vt % 4
            if q == 0:
                stage_ref[0] = outp.tile([128, 2048], F16, tag="ostage",
                                         name="ostage")
            stage = stage_ref[0]
            dst = stage[:, 512 * q:512 * (q + 1)]
            # DVE-only: Act carries the serial cell chain (sig/tanh); the
            # deferred copies land while PE streams the next gates burst
            nc.vector.tensor_copy(dst, pp[:])
            evac_n += 1
            if q == 3:
                ddst = out[128 * (vt - 3):128 * (vt + 1),
                           TOK * kb_prev:TOK * (kb_prev + 1)] \
                    .rearrange("(v p) c -> p v c", v=4)
                nc.sync.dma_start(ddst,
                                  stage[:].rearrange("p (v c) -> p v c", v=4))
        pending.clear()

    hs_prev = None
    # h accessor: encoder final state, layout [u, 32*kc + b]
    h_acc = (lambda kc, hp=hT_enc: hp[:, 32 * kc:32 * (kc + 1)])
    for kb in range(NBLK):
        hs = hsp.tile([128, 4 * TOK], F16, tag="hs", name="hs")
        for st in range(BLK):
            flush_pending()    # prev step's evac+dma land BEFORE cell ops
            xTt = [aT[k][:, 32 * (BLK * kb + st):32 * (BLK * kb + st) + 32]
                   for k in range(2)]
            hd3 = hs[:].rearrange("p (k c) -> p k c", k=4)[:, :, 32 * st:
                                                          32 * (st + 1)]
            c = step(qbT, xTt, wihA, whh_d, h_acc, c, hd3)
            # h(t) now lives at hs cols (512*kc + 32*st + b)
            h_acc = (lambda kc, hp=hs, s=st: hp[:, 512 * kc + 32 * s:
                                                512 * kc + 32 * s + 32])
            if hs_prev is not None:
                proj_mm(hs_prev, kb - 1, 2 * st)
                proj_mm(hs_prev, kb - 1, 2 * st + 1)
            else:
                emit_a_transpose()
                emit_a_transpose()
                emit_a_transpose()
        hs_prev = hs
    # tail: last block's projection
    for vt in range(NVT):
        proj_mm(hs_prev, NBLK - 1, vt)
        if vt % 2 == 1:
            flush_pending()
    flush_pending()


def kernel(**inputs):
    inp = {k: np.asarray(v) for k, v in inputs.items()}
    if "prog" not in _cache:
        _cache["prog"] = build_program()
    nc = _cache["prog"]

    perm = _gate_perm()
    f16 = np.float16

    def tile_bias(bias_perm, n):
        # [n*128] -> [128, n*32]: out[p, 32*mt + b] = bias[128*mt + p]
        return np.repeat(bias_perm.reshape(n, 128).T[:, :, None], 32,
                         axis=2).reshape(128, n * 32).astype(f16)

    def prep_lstm(w_ih, w_hh, b_ih, b_hh):
        wihT = np.ascontiguousarray(w_ih.T[:, perm]).astype(f16)
        whhT = np.ascontiguousarray(w_hh.T[:, perm]).astype(f16)
        bias = (b_ih + b_hh)[perm]
        return wihT, whhT, tile_bias(bias, NMT)

    wihT_e, whhT_e, beT = prep_lstm(inp["q_lstm_w_ih"], inp["q_lstm_w_hh"],
                                    inp["q_lstm_b_ih"], inp["q_lstm_b_hh"])
    wihT_d, whhT_d, bdT = prep_lstm(inp["a_lstm_w_ih"], inp["a_lstm_w_hh"],
                                    inp["a_lstm_b_ih"], inp["a_lstm_b_hh"])
    wihAT = np.ascontiguousarray(wihT_d[:EMB])
    wihQT = np.ascontiguousarray(wihT_d[EMB:])

    q_idx = np.zeros(13 * 128, np.int32)
    q_idx[:B * LQ] = inp["question"].T.reshape(-1).astype(np.int32)
    a_idx = inp["answer"][:, :STEPS].T.reshape(-1).astype(np.int32)

    lin_w = inp["lin_w"].astype(np.float32)   # [32000, 512]
    lin_b = inp["lin_b"].astype(np.float32)

    base = {
        "q_idx": q_idx, "a_idx": a_idx,
        "q_emb": inp["q_emb_w"].astype(f16),
        "a_emb": inp["a_emb_w"].astype(f16),
        "wihT_e": wihT_e, "whhT_e": whhT_e, "beT": beT,
        "wihAT": wihAT, "wihQT": wihQT, "whhT_d": whhT_d, "bdT": bdT,
        "qlwT": np.ascontiguousarray(inp["q_lin_w"].T).astype(f16),
        "qlbT": tile_bias(inp["q_lin_b"].astype(np.float64), 2),
        "i128h": np.eye(128, dtype=f16),
    }
    in_maps = []
    for core in range(NCORES):
        m = dict(base)
        sl = lin_w[VSH * core: VSH * (core + 1)]          # [4000, 512]
        slp = np.zeros((VPAD, HID), np.float32)
        slp[:VSH] = sl
        m["lin_wT"] = np.ascontiguousarray(slp.T).astype(f16)
        in_maps.append(m)

    _cache["in_maps"] = in_maps
    res = run_bass_kernel_spmd(nc, in_maps, core_ids=list(range(NCORES)))
    _cache["last_res"] = res

    final = np.empty((B, W_VOCAB, STEPS), np.float32)
    for core in range(NCORES):
        arr = np.asarray(res.results[core]["out"], dtype=np.float32)
        # [4096, 8192] col = 512*kb + 32*st + b; t = 16*kb + st
        arr = arr.reshape(VPAD, NBLK, BLK, B).transpose(3, 0, 1, 2) \
            .reshape(B, VPAD, STEPS)
        final[:, VSH * core:VSH * (core + 1), :] = arr[:, :VSH, :]
    final += lin_b[None, :, None]
    return final


if __name__ == "__main__":
    import reference
    ins = reference.setup_inputs()
    ref = np.asarray(reference.reference(**ins))
    got = kernel(**{k: np.asarray(v) for k, v in ins.items()})
    err = np.abs(got - ref).max() / (np.abs(ref).max() + 1e-12)
    print("max abs err:", np.abs(got - ref).max(), "rel:", err)


def run_traced():
    nc = _cache["prog"]
    return run_bass_kernel_spmd(nc, _cache["in_maps"],
                                core_ids=list(range(NCORES)), trace=True)
